# revision 14
# baseline (speedup 1.0000x reference)
"""Trainium2 Bass kernel for nn_AttentionEinOps (B=2, S=2048, D=1024, N=16, H=64).

Sharding: batch x head-block. Core c handles batch b = c // 4 and heads
[4*(c%4), 4*(c%4)+4).  Each core computes q/k/v projections for its 4 heads,
LayerNorm on q and k, causal (or general-masked) attention, and a partial
output projection (sum over its 4 heads).  The host sums the 4 partial outputs
per batch and concatenates the per-core k/v head slices.

Numerics: all attention-path matmuls use float32r (tf32-class, ~1.6e-4 rel
error, full PE rate at N=512).  Projections are computed transposed
(weights-stationary, lhsT = packed weight pairs so M=128) from full-fp32
inputs, transposed back to [s, h] for LayerNorm (exact fp32 stats via
bn_stats), then re-transposed to [h, s] for attention.  Scores are computed
transposed (S^T[sk, sq]) so the softmax denominator falls out of a
ones-augmented v column in the z^T accumulation; no max-subtraction is needed
because |S| <= 64 after LayerNorm (Cauchy-Schwarz).  Output projection runs
in bf16 (negligible error) accumulating 4 heads per PSUM tile.
"""

import hashlib
from contextlib import ExitStack

import numpy as np
import ml_dtypes

import concourse.bass as bass
import concourse.tile as tile
from concourse import bacc, mybir
from concourse import bass_utils
from concourse.masks import make_identity

B, SQ, SK, D, N, H = 2, 2048, 2048, 1024, 16, 64
EPS = 1e-5
NEG = -1e30
NCORES = 8
CPB = NCORES // B      # cores per batch (4)
HPC = N // CPB         # heads per core (4)
NT = SQ // 128         # 16 row tiles
NCH = D // 128         # 8 contraction chunks
NJ = SQ // 512         # 4 sq column blocks
NI = SK // 128         # 16 sk chunks

BF16 = mybir.dt.bfloat16
F32 = mybir.dt.float32
F32R = mybir.dt.float32r

_PROGRAM_CACHE: dict = {}


def _build_program(plan, n_mtiles, use_bias, use_g1, use_b1, use_g2, use_b2):
    """plan: per j-block list of (i, kind, idx); kind 0=free, 1=causal diag
    (idx = i-4j in 0..3), 2=general mask tile (idx into maskt input)."""
    nc = bacc.Bacc("TRN2", target_bir_lowering=False, debug=False,
                   num_devices=NCORES)

    xqt = nc.dram_tensor("xqt", [D, SQ], F32R, kind="ExternalInput")
    xkt = nc.dram_tensor("xkt", [D, SK], F32R, kind="ExternalInput")
    qw2 = nc.dram_tensor("qw2", [2, D, 128], F32R, kind="ExternalInput")
    kvw = nc.dram_tensor("kvw", [HPC, D, 128], F32R, kind="ExternalInput")
    wo = nc.dram_tensor("wo", [HPC, 128, D], BF16, kind="ExternalInput")
    if use_bias:
        bqc = nc.dram_tensor("bqc", [128, 2], F32, kind="ExternalInput")
        bkvc = nc.dram_tensor("bkvc", [128, HPC], F32, kind="ExternalInput")
    if use_g1:
        g1 = nc.dram_tensor("g1", [H], F32, kind="ExternalInput")
    if use_b1:
        b1 = nc.dram_tensor("b1", [H], F32, kind="ExternalInput")
    if use_g2:
        g2 = nc.dram_tensor("g2", [H], F32, kind="ExternalInput")
    if use_b2:
        b2 = nc.dram_tensor("b2", [H], F32, kind="ExternalInput")
    if n_mtiles:
        maskt = nc.dram_tensor("maskt", [n_mtiles, 128, 512], F32,
                               kind="ExternalInput")

    outp = nc.dram_tensor("outp", [SQ, D], F32, kind="ExternalOutput")
    ko = nc.dram_tensor("ko", [SQ, HPC, H], F32, kind="ExternalOutput")
    vo = nc.dram_tensor("vo", [SQ, HPC, H], F32, kind="ExternalOutput")

    any_causal = any(kind == 1 for col in plan for (_, kind, _) in col)
    ln_ident1 = not (use_g1 or use_b1)

    with tile.TileContext(nc) as tc, ExitStack() as ctx:
        c1 = ctx.enter_context(tc.tile_pool(name="c1", bufs=1))

        # ---- phase-proj constants ----
        qw2_sb = c1.tile([128, 2, NCH, 128], F32R, tag="qw2")
        kvw_sb = c1.tile([128, HPC, NCH, 128], F32R, tag="kvw")
        nc.sync.dma_start(out=qw2_sb,
                          in_=qw2.ap().rearrange("q (c p) m -> p q c m", p=128))
        nc.sync.dma_start(out=kvw_sb,
                          in_=kvw.ap().rearrange("n (c p) m -> p n c m", p=128))
        qTraw = c1.tile([128, 2, SQ], F32R, tag="qTraw")
        kvTraw = c1.tile([128, HPC, SK], F32R, tag="kvTraw")
        scr = c1.tile([128, SQ], F32, tag="scr")  # constant-fill staging
        nc.vector.memset(scr, 0.0)
        ident = c1.tile([128, 128], F32, tag="ident")
        make_identity(nc, ident)
        ident_r = c1.tile([128, 128], F32R, tag="ident_r")
        nc.vector.tensor_copy(ident_r, ident)
        eps_sb = c1.tile([128, 1], F32, tag="eps")
        nc.vector.memset(eps_sb, EPS)
        onesf = c1.tile([128, H], F32, tag="onesf")
        nc.vector.memset(onesf, 1.0)
        if use_bias:
            bqc_sb = c1.tile([128, 2], F32, tag="bqc")
            bkvc_sb = c1.tile([128, HPC], F32, tag="bkvc")
            nc.sync.dma_start(out=bqc_sb, in_=bqc[:])
            nc.sync.dma_start(out=bkvc_sb, in_=bkvc[:])
        if use_g1:
            g1_sb = c1.tile([128, H], F32, tag="g1")
            nc.sync.dma_start(out=g1_sb, in_=g1.ap().to_broadcast([128, H]))
        if use_b1:
            b1_sb = c1.tile([128, H], F32, tag="b1")
            nc.sync.dma_start(out=b1_sb, in_=b1.ap().to_broadcast([128, H]))
        if use_g2:
            g2_sb = c1.tile([128, H], F32, tag="g2")
            nc.sync.dma_start(out=g2_sb, in_=g2.ap().to_broadcast([128, H]))
        if use_b2:
            b2_sb = c1.tile([128, H], F32, tag="b2")
            nc.sync.dma_start(out=b2_sb, in_=b2.ap().to_broadcast([128, H]))

        # ---- phase PROJ: weights-stationary f32r, x streamed by j-blocks ----
        with tc.tile_pool(name="xs", bufs=2) as xs, \
             tc.tile_pool(name="psp", bufs=4, space="PSUM") as psp:
            for j in range(NJ):
                jsl = slice(j * 512, (j + 1) * 512)
                xq_j = xs.tile([128, NCH, 512], F32R, tag="xq_j")
                xk_j = xs.tile([128, NCH, 512], F32R, tag="xk_j")
                nc.sync.dma_start(
                    out=xq_j,
                    in_=xqt[:, jsl].rearrange("(c p) f -> p c f", p=128))
                nc.sync.dma_start(
                    out=xk_j,
                    in_=xkt[:, jsl].rearrange("(c p) f -> p c f", p=128))
                for pr in range(2):
                    ps = psp.tile([128, 512], F32, tag="prj", name="psq")
                    for c in range(NCH):
                        nc.tensor.matmul(ps, qw2_sb[:, pr, c, :], xq_j[:, c, :],
                                         start=(c == 0), stop=(c == NCH - 1))
                    nc.vector.tensor_copy(qTraw[:, pr, jsl], ps)
                for n in range(HPC):
                    ps = psp.tile([128, 512], F32, tag="prj", name="pskv")
                    for c in range(NCH):
                        nc.tensor.matmul(ps, kvw_sb[:, n, c, :], xk_j[:, c, :],
                                         start=(c == 0), stop=(c == NCH - 1))
                    nc.vector.tensor_copy(kvTraw[:, n, jsl], ps)

        if use_bias:
            for pr in range(2):
                nc.vector.tensor_scalar_add(
                    out=qTraw[:, pr, :], in0=qTraw[:, pr, :],
                    scalar1=bqc_sb[:, pr:pr + 1])
            for n in range(HPC):
                nc.vector.tensor_scalar_add(
                    out=kvTraw[:, n, :], in0=kvTraw[:, n, :],
                    scalar1=bkvc_sb[:, n:n + 1])

        # ---- attention-phase persistent tiles ----
        c2 = ctx.enter_context(tc.tile_pool(name="c2", bufs=1))
        raws = ctx.enter_context(tc.tile_pool(name="raws", bufs=2))
        stats = ctx.enter_context(tc.tile_pool(name="stats", bufs=2))
        lnt = ctx.enter_context(tc.tile_pool(name="lnt", bufs=3))
        ppool = ctx.enter_context(tc.tile_pool(name="ppool", bufs=3))
        smt = ctx.enter_context(tc.tile_pool(name="smt", bufs=3))
        bcp = ctx.enter_context(tc.tile_pool(name="bcp", bufs=2))
        osb = ctx.enter_context(tc.tile_pool(name="osb", bufs=3))
        ps_tr = ctx.enter_context(
            tc.tile_pool(name="ps_tr", bufs=3, space="PSUM"))
        ps_big = ctx.enter_context(
            tc.tile_pool(name="ps_big", bufs=4, space="PSUM"))

        wo_sb = c2.tile([128, HPC, D], BF16, tag="wo")
        nc.sync.dma_start(out=wo_sb, in_=wo.ap().rearrange("n p d -> p n d"))
        qTn_bufs = [c2.tile([128, SQ], F32R, tag=f"qTn{i}", name=f"qTn{i}")
                    for i in range(2)]
        kTn_bufs = [c2.tile([128, SK], F32R, tag=f"kTn{i}", name=f"kTn{i}")
                    for i in range(2)]
        for t_ in qTn_bufs + kTn_bufs:
            nc.vector.tensor_copy(t_[64:128, :], scr[64:128, :])
        rd_bufs = [c2.tile([128, 512], F32R, tag=f"rd{i}", name=f"rd{i}")
                   for i in range(2)]
        for t_ in rd_bufs:
            nc.vector.tensor_copy(t_, scr[:, 0:512])
        ones_r = c2.tile([128, H], F32R, tag="ones_r")
        nc.vector.tensor_copy(ones_r, scr[:, 0:H])
        nc.vector.tensor_copy(ones_r[0:1, :], onesf[0:1, 0:H])
        zTn = c2.tile([128, HPC, SQ], BF16, tag="zTn")
        nc.gpsimd.memset(zTn, 0.0)

        if any_causal:
            cm = c2.tile([128, 4, 512], F32, tag="cm")
            nc.gpsimd.memset(cm, 0.0)
            for oi in range(4):
                # additive mask: 0.0 where f - p - 128*oi >= 0 else -1e30
                nc.gpsimd.affine_select(
                    out=cm[:, oi, :], in_=cm[:, oi, :],
                    compare_op=mybir.AluOpType.is_ge, fill=NEG,
                    base=-128 * oi, channel_multiplier=-1,
                    pattern=[[1, 512]],
                )
        if n_mtiles:
            mk_sb = c2.tile([128, n_mtiles, 512], F32, tag="mk")
            nc.sync.dma_start(out=mk_sb,
                              in_=maskt.ap().rearrange("m p f -> p m f"))

        # ---- per-head: transpose back, LayerNorm, re-transpose, attention ----
        for hl in range(HPC):
            pr, sub = hl // 2, hl % 2
            qsrc = qTraw[sub * 64:(sub + 1) * 64, pr, :]
            ksrc = kvTraw[0:64, hl, :]
            vsrc = kvTraw[64:128, hl, :]
            qTn = qTn_bufs[hl % 2]
            kTn = kTn_bufs[hl % 2]

            q_raw = raws.tile([128, NT, H], F32, tag="q_raw")
            k_raw = raws.tile([128, NT, H], F32, tag="k_raw")
            v_aug = raws.tile([128, NT, H + 1], F32R, tag="v_aug")
            nc.vector.tensor_copy(v_aug[:, :, H:H + 1], onesf[:, 0:NT])
            mv_q = stats.tile([128, NT, 2], F32, tag="mv_q")
            mv_k = stats.tile([128, NT, 2], F32, tag="mv_k")
            sd_q = stats.tile([128, NT], F32, tag="sd_q")
            sd_k = stats.tile([128, NT], F32, tag="sd_k")

            # transpose raw projections back to [s, h] + LN stats
            for t in range(NT):
                tsl = slice(t * 128, (t + 1) * 128)
                pq = ps_tr.tile([128, 128], F32R, tag="pst", name="pq")
                nc.tensor.transpose(pq[:, 0:64], qsrc[:, tsl],
                                    ident_r[sub * 64:(sub + 1) * 64,
                                            sub * 64:(sub + 1) * 64])
                nc.vector.tensor_copy(q_raw[:, t, :], pq[:, 0:64])
                pk = ps_tr.tile([128, 128], F32R, tag="pst", name="pk")
                nc.tensor.transpose(pk[:, 0:64], ksrc[:, tsl], ident_r[0:64, 0:64])
                nc.vector.tensor_copy(k_raw[:, t, :], pk[:, 0:64])
                pv = ps_tr.tile([128, 128], F32R, tag="pst", name="pv")
                nc.tensor.transpose(pv[:, 0:64], vsrc[:, tsl], ident_r[64:128, 64:128])
                nc.vector.tensor_copy(v_aug[:, t, 0:H], pv[:, 0:64])
                stq = lnt.tile([128, 6], F32, tag="stq")
                stk = lnt.tile([128, 6], F32, tag="stk")
                nc.vector.bn_stats(stq, q_raw[:, t, :])
                nc.vector.bn_aggr(mv_q[:, t, :], stq)
                nc.vector.bn_stats(stk, k_raw[:, t, :])
                nc.vector.bn_aggr(mv_k[:, t, :], stk)

            # v output DMA (f32r bits are valid f32)
            nc.sync.dma_start(
                out=vo.ap()[:, hl, :].rearrange("(t p) h -> p t h", p=128),
                in_=v_aug[:, :, 0:H].bitcast(F32))

            # batched rstd = 1/sqrt(var + eps)
            nc.scalar.activation(sd_q, mv_q[:, :, 1],
                                 mybir.ActivationFunctionType.Sqrt, bias=eps_sb)
            nc.vector.reciprocal(sd_q, sd_q)
            nc.scalar.activation(sd_k, mv_k[:, :, 1],
                                 mybir.ActivationFunctionType.Sqrt, bias=eps_sb)
            nc.vector.reciprocal(sd_k, sd_k)

            # apply LN + transpose to [h, s]
            for t in range(NT):
                tsl = slice(t * 128, (t + 1) * 128)
                q_ln = lnt.tile([128, H], F32R if ln_ident1 else F32,
                                tag="q_ln", name="q_ln")
                nc.vector.tensor_scalar(
                    out=q_ln, in0=q_raw[:, t, :],
                    scalar1=mv_q[:, t, 0:1], scalar2=sd_q[:, t:t + 1],
                    op0=mybir.AluOpType.subtract, op1=mybir.AluOpType.mult)
                if use_g1:
                    nc.vector.tensor_mul(q_ln, q_ln, g1_sb)
                if use_b1:
                    nc.vector.tensor_add(q_ln, q_ln, b1_sb)
                pst = ps_tr.tile([128, 128], F32R if ln_ident1 else F32, tag="pst", name="pstq")
                nc.tensor.transpose(pst[0:64, :], q_ln,
                                    ident_r if ln_ident1 else ident)
                nc.vector.tensor_copy(qTn[0:64, tsl], pst[0:64, :])

                # k LN in place (k_raw becomes k_ln, DMA'd out as k)
                nc.vector.tensor_scalar(
                    out=k_raw[:, t, :], in0=k_raw[:, t, :],
                    scalar1=mv_k[:, t, 0:1], scalar2=sd_k[:, t:t + 1],
                    op0=mybir.AluOpType.subtract, op1=mybir.AluOpType.mult)
                if use_g2:
                    nc.vector.tensor_mul(k_raw[:, t, :], k_raw[:, t, :], g2_sb)
                if use_b2:
                    nc.vector.tensor_add(k_raw[:, t, :], k_raw[:, t, :], b2_sb)
                pst2 = ps_tr.tile([128, 128], F32, tag="pst", name="pstk")
                nc.tensor.transpose(pst2[0:64, :], k_raw[:, t, :], ident)
                nc.vector.tensor_copy(kTn[0:64, tsl], pst2[0:64, :])

            nc.sync.dma_start(
                out=ko.ap()[:, hl, :].rearrange("(t p) h -> p t h", p=128),
                in_=k_raw)

            # attention: S^T tiles -> exp -> z^T accumulation with denominator
            for j in range(NJ):
                col = plan[j]
                if not col:
                    continue
                jsl = slice(j * 512, (j + 1) * 512)
                zt = ps_big.tile([128, 512], F32, tag="big", name="zt")
                for idx, (i, kind, midx) in enumerate(col):
                    st_ps = ps_big.tile([128, 512], F32, tag="big",
                                        name="st_ps")
                    nc.tensor.matmul(
                        st_ps, kTn[:, i * 128:(i + 1) * 128], qTn[:, jsl],
                        start=True, stop=True)
                    p = ppool.tile([128, 512], F32R, tag="p", name="p")
                    if kind == 0:
                        nc.scalar.activation(p, st_ps,
                                             mybir.ActivationFunctionType.Exp)
                    else:
                        mt = cm[:, midx, :] if kind == 1 else mk_sb[:, midx, :]
                        sm = smt.tile([128, 512], F32, tag="sm", name="sm")
                        nc.vector.tensor_add(sm, st_ps, mt)
                        nc.scalar.activation(p, sm,
                                             mybir.ActivationFunctionType.Exp)
                    nc.tensor.matmul(
                        zt[0:H + 1, :], v_aug[:, i, :], p,
                        start=(idx == 0), stop=(idx == len(col) - 1))
                # denominator -> reciprocal -> broadcast -> normalize
                rd = rd_bufs[(hl * NJ + j) % 2]
                with nc.allow_low_precision(
                        reason="f32r reciprocal feeding f32r matmul"):
                    nc.vector.reciprocal(rd[0:1, :], zt[H:H + 1, :])
                bc_ps = ps_big.tile([64, 512], F32, tag="big", name="bc_ps")
                nc.tensor.matmul(bc_ps, ones_r, rd, start=True, stop=True)
                bc_sb = bcp.tile([64, 512], F32, tag="bc_sb")
                nc.vector.tensor_copy(bc_sb, bc_ps)
                nc.vector.tensor_tensor(
                    out=zTn[0:64, hl, jsl], in0=zt[0:64, :], in1=bc_sb,
                    op=mybir.AluOpType.mult)

        # ---- output projection (4 heads accumulated per PSUM tile) ----
        for t in range(NT):
            for dh in range(2):
                ot = ps_big.tile([128, 512], F32, tag="big", name="ot")
                for hl in range(HPC):
                    nc.tensor.matmul(
                        ot, zTn[:, hl, t * 128:(t + 1) * 128],
                        wo_sb[:, hl, dh * 512:(dh + 1) * 512],
                        start=(hl == 0), stop=(hl == HPC - 1))
                o_sb = osb.tile([128, 512], F32, tag="o_sb")
                nc.vector.tensor_copy(o_sb, ot)
                nc.sync.dma_start(
                    out=outp[t * 128:(t + 1) * 128, dh * 512:(dh + 1) * 512],
                    in_=o_sb)

    nc.compile()
    return nc


def _make_plan(mask):
    """Classify [sk_chunk=128 x sq_block=512] tiles of the transposed mask."""
    mask = np.asarray(mask, dtype=bool)
    causal = np.array_equal(mask, np.triu(np.ones((SQ, SK), dtype=bool), k=1))
    plan, mtiles = [], []
    if causal:
        for j in range(NJ):
            col = []
            for i in range(NI):
                if i < 4 * j:
                    col.append((i, 0, 0))
                elif i <= 4 * j + 3:
                    col.append((i, 1, i - 4 * j))
            plan.append(col)
        return plan, mtiles
    mt = mask.T  # [sk, sq], True = masked
    for j in range(NJ):
        col = []
        for i in range(NI):
            sub = mt[i * 128:(i + 1) * 128, j * 512:(j + 1) * 512]
            if sub.all():
                continue
            if not sub.any():
                col.append((i, 0, 0))
            else:
                mtiles.append(np.where(sub, np.float32(NEG), np.float32(0.0)))
                col.append((i, 2, len(mtiles) - 1))
        plan.append(col)
    return plan, mtiles


def _get_program(plan, n_mtiles, use_bias, use_g1, use_b1, use_g2, use_b2):
    key = hashlib.sha256(
        repr((plan, n_mtiles, use_bias, use_g1, use_b1, use_g2,
              use_b2)).encode()).hexdigest()
    if key not in _PROGRAM_CACHE:
        _PROGRAM_CACHE[key] = _build_program(
            plan, n_mtiles, use_bias, use_g1, use_b1, use_g2, use_b2)
    return _PROGRAM_CACHE[key]


def kernel(x_q, x_kv, mask, W_Q, W_K, W_V, W_O, b_Q, b_K, b_V, b_O,
           ln1_g, ln1_b, ln2_g, ln2_b):
    x_q = np.asarray(x_q, dtype=np.float32)
    x_kv = np.asarray(x_kv, dtype=np.float32)
    W_Q = np.asarray(W_Q, dtype=np.float32)
    W_K = np.asarray(W_K, dtype=np.float32)
    W_V = np.asarray(W_V, dtype=np.float32)
    W_O = np.asarray(W_O, dtype=np.float32)
    b_Q = np.asarray(b_Q, dtype=np.float32)
    b_K = np.asarray(b_K, dtype=np.float32)
    b_V = np.asarray(b_V, dtype=np.float32)
    b_O = np.asarray(b_O, dtype=np.float32)
    ln1_g = np.asarray(ln1_g, dtype=np.float32)
    ln1_b = np.asarray(ln1_b, dtype=np.float32)
    ln2_g = np.asarray(ln2_g, dtype=np.float32)
    ln2_b = np.asarray(ln2_b, dtype=np.float32)

    plan, mtiles = _make_plan(mask)
    n_mtiles = len(mtiles)
    use_bias = bool(np.any(b_Q) or np.any(b_K) or np.any(b_V))
    use_g1 = not np.all(ln1_g == 1.0)
    use_b1 = bool(np.any(ln1_b))
    use_g2 = not np.all(ln2_g == 1.0)
    use_b2 = bool(np.any(ln2_b))

    nc = _get_program(tuple(map(tuple, plan)), n_mtiles,
                      use_bias, use_g1, use_b1, use_g2, use_b2)

    bf = ml_dtypes.bfloat16
    xqt_b = [np.ascontiguousarray(x_q[b].T) for b in range(B)]
    xkt_b = [np.ascontiguousarray(x_kv[b].T) for b in range(B)]
    wo_pad = np.zeros((N, 128, D), dtype=np.float32)
    wo_pad[:, :H, :] = W_O
    wo_pad = wo_pad.astype(bf)
    if n_mtiles:
        maskt_arr = np.ascontiguousarray(np.stack(mtiles))

    in_maps = []
    for core in range(NCORES):
        b = core // CPB
        n0 = (core % CPB) * HPC
        qw2_arr = np.empty((2, D, 128), dtype=np.float32)
        kvw_arr = np.empty((HPC, D, 128), dtype=np.float32)
        for pr in range(2):
            qw2_arr[pr, :, 0:64] = W_Q[n0 + 2 * pr]
            qw2_arr[pr, :, 64:128] = W_Q[n0 + 2 * pr + 1]
        for n in range(HPC):
            kvw_arr[n, :, 0:64] = W_K[n0 + n]
            kvw_arr[n, :, 64:128] = W_V[n0 + n]
        m = {
            "xqt": xqt_b[b],
            "xkt": xkt_b[b],
            "qw2": qw2_arr,
            "kvw": kvw_arr,
            "wo": np.ascontiguousarray(wo_pad[n0:n0 + HPC]),
        }
        if use_bias:
            bqc_arr = np.empty((128, 2), dtype=np.float32)
            bkvc_arr = np.empty((128, HPC), dtype=np.float32)
            for pr in range(2):
                bqc_arr[0:64, pr] = b_Q[n0 + 2 * pr]
                bqc_arr[64:128, pr] = b_Q[n0 + 2 * pr + 1]
            for n in range(HPC):
                bkvc_arr[0:64, n] = b_K[n0 + n]
                bkvc_arr[64:128, n] = b_V[n0 + n]
            m["bqc"] = bqc_arr
            m["bkvc"] = bkvc_arr
        if use_g1:
            m["g1"] = ln1_g
        if use_b1:
            m["b1"] = ln1_b
        if use_g2:
            m["g2"] = ln2_g
        if use_b2:
            m["b2"] = ln2_b
        if n_mtiles:
            m["maskt"] = maskt_arr
        in_maps.append(m)

    res = bass_utils.run_bass_kernel_spmd(nc, in_maps,
                                          core_ids=list(range(NCORES)))

    out = np.zeros((B, SQ, D), dtype=np.float32)
    k_full = np.empty((B, SQ, N, H), dtype=np.float32)
    v_full = np.empty((B, SQ, N, H), dtype=np.float32)
    for core in range(NCORES):
        b = core // CPB
        n0 = (core % CPB) * HPC
        r = res.results[core]
        out[b] += r["outp"]
        k_full[b][:, n0:n0 + HPC, :] = r["ko"]
        v_full[b][:, n0:n0 + HPC, :] = r["vo"]
    out += b_O
    return out, k_full, v_full


# revision 25
# speedup vs baseline: 1.1027x; 1.1027x over previous
"""Trainium2 Bass kernel for nn_AttentionEinOps (B=2, S=2048, D=1024, N=16, H=64).

Sharding: batch x head-block. Core c handles batch b = c // 4 and heads
[4*(c%4), 4*(c%4)+4).  Each core computes q/k/v projections for its 4 heads,
LayerNorm on q and k, causal (or general-masked) attention, and a partial
output projection (sum over its 4 heads).  The host sums the 4 partial outputs
per batch and concatenates the per-core k/v head slices.

Numerics: all attention-path matmuls use float32r (tf32-class, ~1.6e-4 rel
error, full PE rate at N=512).  Projections are computed transposed
(weights-stationary, lhsT = packed weight pairs so M=128) from full-fp32
inputs, transposed back to [s, h] for LayerNorm (exact fp32 stats via
bn_stats), then re-transposed to [h, s] for attention.  Scores are computed
transposed (S^T[sk, sq]) so the softmax denominator falls out of a
ones-augmented v column in the z^T accumulation; no max-subtraction is needed
because |S| <= 64 after LayerNorm (Cauchy-Schwarz).  Output projection runs
in bf16 (negligible error) accumulating 4 heads per PSUM tile.
"""

import hashlib
from contextlib import ExitStack

import numpy as np
import ml_dtypes

import concourse.bass as bass
import concourse.tile as tile
from concourse import bacc, mybir
from concourse import bass_utils
from concourse.masks import make_identity

B, SQ, SK, D, N, H = 2, 2048, 2048, 1024, 16, 64
EPS = 1e-5
NEG = -1e30
NCORES = 8
CPB = NCORES // B      # cores per batch (4)
HPC = N // CPB         # heads per core (4)
NT = SQ // 128         # 16 row tiles
NCH = D // 128         # 8 contraction chunks
NJ = SQ // 512         # 4 sq column blocks
NI = SK // 128         # 16 sk chunks

BF16 = mybir.dt.bfloat16
F32 = mybir.dt.float32
F32R = mybir.dt.float32r
I32 = mybir.dt.int32

_PROGRAM_CACHE: dict = {}


def _build_program(plan, n_mtiles, use_bias, use_g1, use_b1, use_g2, use_b2):
    """plan: per j-block list of (i, kind, idx); kind 0=free, 1=causal diag
    (idx = i-4j in 0..3), 2=general mask tile (idx into maskt input)."""
    nc = bacc.Bacc("TRN2", target_bir_lowering=False, debug=False,
                   num_devices=NCORES)

    xqt = nc.dram_tensor("xqt", [D, SQ], F32R, kind="ExternalInput")
    xkt = nc.dram_tensor("xkt", [D, SK], F32R, kind="ExternalInput")
    qw2 = nc.dram_tensor("qw2", [2, D, 128], F32R, kind="ExternalInput")
    kvw = nc.dram_tensor("kvw", [HPC, D, 128], F32R, kind="ExternalInput")
    wo2 = nc.dram_tensor("wo2", [2, 128, D], BF16, kind="ExternalInput")
    if use_bias:
        bqc = nc.dram_tensor("bqc", [128, 2], F32, kind="ExternalInput")
        bkvc = nc.dram_tensor("bkvc", [128, HPC], F32, kind="ExternalInput")
    if use_g1:
        g1 = nc.dram_tensor("g1", [H], F32, kind="ExternalInput")
    if use_b1:
        b1 = nc.dram_tensor("b1", [H], F32, kind="ExternalInput")
    if use_g2:
        g2 = nc.dram_tensor("g2", [H], F32, kind="ExternalInput")
    if use_b2:
        b2 = nc.dram_tensor("b2", [H], F32, kind="ExternalInput")
    if n_mtiles:
        maskt = nc.dram_tensor("maskt", [n_mtiles, 128, 512], F32,
                               kind="ExternalInput")

    outp = nc.dram_tensor("outp", [SQ, D], F32, kind="ExternalOutput")
    ko = nc.dram_tensor("ko", [SQ, HPC, H], F32, kind="ExternalOutput")
    vo = nc.dram_tensor("vo", [SQ, HPC, H], F32, kind="ExternalOutput")

    any_causal = any(kind == 1 for col in plan for (_, kind, _) in col)
    ln_ident1 = not (use_g1 or use_b1)

    with tile.TileContext(nc) as tc, ExitStack() as ctx:
        c1 = ctx.enter_context(tc.tile_pool(name="c1", bufs=1))

        # ---- phase-proj constants ----
        qw2_sb = c1.tile([128, 2, NCH, 128], F32R, tag="qw2")
        kvw_sb = c1.tile([128, HPC, NCH, 128], F32R, tag="kvw")
        nc.sync.dma_start(out=qw2_sb,
                          in_=qw2.ap().rearrange("q (c p) m -> p q c m", p=128))
        nc.sync.dma_start(out=kvw_sb,
                          in_=kvw.ap().rearrange("n (c p) m -> p n c m", p=128))
        qTraw_j = [c1.tile([128, 2, 512], F32R, tag=f"qTraw{j}",
                           name=f"qTraw{j}") for j in range(NJ)]
        kvTraw_j = [c1.tile([128, HPC, 512], F32R, tag=f"kvTraw{j}",
                            name=f"kvTraw{j}") for j in range(NJ)]
        scr = c1.tile([128, 512], F32, tag="scr")  # constant-fill staging
        nc.vector.memset(scr, 0.0)
        ident = c1.tile([128, 128], F32, tag="ident")
        make_identity(nc, ident)
        ident_r = c1.tile([128, 128], F32R, tag="ident_r")
        nc.vector.tensor_copy(ident_r, ident)
        eps_sb = c1.tile([128, 1], F32, tag="eps")
        nc.vector.memset(eps_sb, EPS)
        onesf = c1.tile([128, H], F32, tag="onesf")
        nc.vector.memset(onesf, 1.0)
        if use_bias:
            bqc_sb = c1.tile([128, 2], F32, tag="bqc")
            bkvc_sb = c1.tile([128, HPC], F32, tag="bkvc")
            nc.sync.dma_start(out=bqc_sb, in_=bqc[:])
            nc.sync.dma_start(out=bkvc_sb, in_=bkvc[:])
        if use_g1:
            g1_sb = c1.tile([128, H], F32, tag="g1")
            nc.sync.dma_start(out=g1_sb, in_=g1.ap().to_broadcast([128, H]))
        if use_b1:
            b1_sb = c1.tile([128, H], F32, tag="b1")
            nc.sync.dma_start(out=b1_sb, in_=b1.ap().to_broadcast([128, H]))
        if use_g2:
            g2_sb = c1.tile([128, H], F32, tag="g2")
            nc.sync.dma_start(out=g2_sb, in_=g2.ap().to_broadcast([128, H]))
        if use_b2:
            b2_sb = c1.tile([128, H], F32, tag="b2")
            nc.sync.dma_start(out=b2_sb, in_=b2.ap().to_broadcast([128, H]))

        # ---- attention-phase pools (before proj so emission can interleave) ----
        c2 = ctx.enter_context(tc.tile_pool(name="c2", bufs=1))
        raws = ctx.enter_context(tc.tile_pool(name="raws", bufs=2))
        stats = ctx.enter_context(tc.tile_pool(name="stats", bufs=2))
        lnt = ctx.enter_context(tc.tile_pool(name="lnt", bufs=2))
        ppool = ctx.enter_context(tc.tile_pool(name="ppool", bufs=3))
        smt = ctx.enter_context(tc.tile_pool(name="smt", bufs=3))
        bcp = ctx.enter_context(tc.tile_pool(name="bcp", bufs=2))
        osb = ctx.enter_context(tc.tile_pool(name="osb", bufs=3))
        ps_tr = ctx.enter_context(
            tc.tile_pool(name="ps_tr", bufs=3, space="PSUM"))
        ps_big = ctx.enter_context(
            tc.tile_pool(name="ps_big", bufs=4, space="PSUM"))

        wo_sb = c2.tile([128, 2, D], BF16, tag="wo")
        nc.sync.dma_start(out=wo_sb, in_=wo2.ap().rearrange("q p d -> p q d"))
        qTn_bufs = [c2.tile([128, SQ], F32R, tag=f"qTn{i}", name=f"qTn{i}")
                    for i in range(2)]
        kTn_bufs = [c2.tile([128, SK], F32R, tag=f"kTn{i}", name=f"kTn{i}")
                    for i in range(2)]
        for t_ in qTn_bufs + kTn_bufs:
            for qq in range(4):
                nc.vector.tensor_copy(t_[64:128, qq * 512:(qq + 1) * 512],
                                      scr[64:128, 0:512])
        rd_bufs = [c2.tile([1, 512], F32, tag=f"rd{i}", name=f"rd{i}")
                   for i in range(2)]
        magic_sb = c2.tile([128, NT], I32, tag="magic")
        nc.vector.memset(magic_sb, 0x5f3759df)
        zTn_j = [c2.tile([128, 2, 512], BF16, tag=f"zTn{j}",
                         name=f"zTn{j}") for j in range(NJ)]
        for t_ in zTn_j:
            nc.gpsimd.memset(t_, 0.0)
        if n_mtiles:
            mk_sb = c2.tile([128, n_mtiles, 512], F32, tag="mk")
            nc.sync.dma_start(out=mk_sb,
                              in_=maskt.ap().rearrange("m p f -> p m f"))

        hstate = {}

        def emit_alloc(hl):
            q_raw = raws.tile([128, NT, H], F32, tag="q_raw",
                              name=f"q_raw{hl}")
            k_raw = raws.tile([128, NT, H], F32, tag="k_raw",
                              name=f"k_raw{hl}")
            v_aug = raws.tile([128, NT, H + 1], F32R, tag="v_aug",
                              name=f"v_aug{hl}")
            nc.vector.tensor_copy(v_aug[:, :, H:H + 1], onesf[:, 0:NT])
            hstate[hl] = dict(
                q_raw=q_raw, k_raw=k_raw, v_aug=v_aug,
                st6_q=stats.tile([128, NT, 6], F32, tag="st6_q",
                                 name=f"st6_q{hl}"),
                st6_k=stats.tile([128, NT, 6], F32, tag="st6_k",
                                 name=f"st6_k{hl}"),
                mn_q=stats.tile([128, NT], F32, tag="mn_q", name=f"mn_q{hl}"),
                mn_k=stats.tile([128, NT], F32, tag="mn_k", name=f"mn_k{hl}"),
                sd_q=stats.tile([128, NT], F32, tag="sd_q", name=f"sd_q{hl}"),
                sd_k=stats.tile([128, NT], F32, tag="sd_k", name=f"sd_k{hl}"),
                tmp_s=stats.tile([128, NT], F32, tag="tmp_s",
                                 name=f"tmp_s{hl}"),
                vv=stats.tile([128, NT], F32, tag="vv", name=f"vv{hl}"),
                hh=stats.tile([128, NT], F32, tag="hh", name=f"hh{hl}"),
                yy=stats.tile([128, NT], F32, tag="yy", name=f"yy{hl}"),
                t1=stats.tile([128, NT], F32, tag="t1", name=f"t1{hl}"),
            )

        def emit_raw(hl, ts):
            """Transpose raw projections of head hl back to [s, h]."""
            if hl not in hstate:
                emit_alloc(hl)
            hs = hstate[hl]
            pr, sub = hl // 2, hl % 2
            ts = list(ts)
            idq = ident_r[sub * 64:(sub + 1) * 64, sub * 64:(sub + 1) * 64]
            for dst, idnt, srcf in (
                ("q_raw", idq,
                 lambda jb, osl: qTraw_j[jb][sub * 64:(sub + 1) * 64, pr, osl]),
                ("k_raw", ident_r[0:64, 0:64],
                 lambda jb, osl: kvTraw_j[jb][0:64, hl, osl]),
                ("v_aug", ident_r[64:128, 64:128],
                 lambda jb, osl: kvTraw_j[jb][64:128, hl, osl]),
            ):
                for g0 in range(0, len(ts), 8):
                    grp = ts[g0:g0 + 8]
                    p8 = ps_tr.tile([128, 512], F32R, tag="pst", name="p8")
                    for k_, t in enumerate(grp):
                        jb, off = t // 4, (t % 4) * 128
                        osl = slice(off, off + 128)
                        nc.tensor.matmul(
                            p8[:, 64 * k_:64 * k_ + 64], srcf(jb, osl), idnt,
                            is_transpose=True, start=(k_ == 0),
                            stop=(k_ == len(grp) - 1))
                    if dst == "v_aug":
                        nc.scalar.copy(
                            hs[dst][:, grp[0]:grp[0] + len(grp), 0:H],
                            p8[:, 0:64 * len(grp)])
                    else:
                        nc.vector.tensor_copy(
                            hs[dst][:, grp[0]:grp[0] + len(grp), :],
                            p8[:, 0:64 * len(grp)])

        def emit_saf(hl):
            """LayerNorm stats + apply + forward transposes + k/v DMAs."""
            hs = hstate[hl]
            pr, sub = hl // 2, hl % 2
            qTn = qTn_bufs[hl % 2]
            kTn = kTn_bufs[hl % 2]
            q_raw, k_raw, v_aug = hs["q_raw"], hs["k_raw"], hs["v_aug"]
            nc.sync.dma_start(
                out=vo.ap()[:, hl, :].rearrange("(t p) h -> p t h", p=128),
                in_=v_aug[:, :, 0:H].bitcast(F32))
            # batched stats; merge bn_stats even/odd halves via Chan:
            # var = (M2e + M2o + 16*(me-mo)^2) / 64;  mean = (me + mo) / 2
            for which in ("q", "k"):
                st6 = hs[f"st6_{which}"]
                mn = hs[f"mn_{which}"]
                sd = hs[f"sd_{which}"]
                tmp_s = hs["tmp_s"]
                src = q_raw if which == "q" else k_raw
                for t in range(NT):
                    nc.vector.bn_stats(st6[:, t, :], src[:, t, :])
                me, mo = st6[:, :, 1], st6[:, :, 4]
                m2e, m2o = st6[:, :, 2], st6[:, :, 5]
                nc.vector.tensor_tensor(out=mn, in0=me, in1=mo,
                                        op=mybir.AluOpType.add)
                nc.vector.tensor_scalar_mul(out=mn, in0=mn, scalar1=0.5)
                nc.vector.tensor_tensor(out=sd, in0=me, in1=mo,
                                        op=mybir.AluOpType.subtract)
                nc.vector.scalar_tensor_tensor(
                    out=sd, in0=sd, scalar=16.0, in1=sd,
                    op0=mybir.AluOpType.mult, op1=mybir.AluOpType.mult)
                nc.vector.tensor_tensor(out=tmp_s, in0=m2e, in1=m2o,
                                        op=mybir.AluOpType.add)
                nc.vector.tensor_tensor(out=sd, in0=sd, in1=tmp_s,
                                        op=mybir.AluOpType.add)
                # rstd = 1/sqrt(sd/64 + eps) via quake + 2 Newton iters (DVE)
                vv, hh, yy, t1 = hs["vv"], hs["hh"], hs["yy"], hs["t1"]
                nc.vector.tensor_scalar(
                    out=vv, in0=sd, scalar1=1.0 / 64.0, scalar2=EPS,
                    op0=mybir.AluOpType.mult, op1=mybir.AluOpType.add)
                nc.vector.tensor_scalar_mul(out=hh, in0=vv, scalar1=0.5)
                nc.vector.tensor_scalar(
                    out=yy.bitcast(I32), in0=vv.bitcast(I32), scalar1=1,
                    scalar2=None, op0=mybir.AluOpType.arith_shift_right)
                nc.vector.tensor_tensor(
                    out=yy.bitcast(I32), in0=magic_sb, in1=yy.bitcast(I32),
                    op=mybir.AluOpType.subtract)
                for _ in range(3):
                    nc.vector.tensor_tensor(out=t1, in0=yy, in1=yy,
                                            op=mybir.AluOpType.mult)
                    nc.vector.tensor_tensor(out=t1, in0=t1, in1=yy,
                                            op=mybir.AluOpType.mult)
                    nc.vector.tensor_tensor(out=t1, in0=t1, in1=hh,
                                            op=mybir.AluOpType.mult)
                    nc.vector.scalar_tensor_tensor(
                        out=yy, in0=yy, scalar=1.5, in1=t1,
                        op0=mybir.AluOpType.mult,
                        op1=mybir.AluOpType.subtract)
                nc.vector.tensor_copy(sd, yy)
            # batched LN apply (broadcast mean/rstd along h)
            q_lnb = lnt.tile([128, NT, H], F32R if ln_ident1 else F32,
                             tag="q_lnb", name=f"q_lnb{hl}")
            nc.vector.tensor_tensor(
                out=q_lnb, in0=q_raw,
                in1=hs["mn_q"].unsqueeze(2).broadcast_to([128, NT, H]),
                op=mybir.AluOpType.subtract)
            nc.vector.tensor_tensor(
                out=q_lnb, in0=q_lnb,
                in1=hs["sd_q"].unsqueeze(2).broadcast_to([128, NT, H]),
                op=mybir.AluOpType.mult)
            nc.vector.tensor_tensor(
                out=k_raw, in0=k_raw,
                in1=hs["mn_k"].unsqueeze(2).broadcast_to([128, NT, H]),
                op=mybir.AluOpType.subtract)
            nc.vector.tensor_tensor(
                out=k_raw, in0=k_raw,
                in1=hs["sd_k"].unsqueeze(2).broadcast_to([128, NT, H]),
                op=mybir.AluOpType.mult)
            # forward transposes to [h, s], 4 per PSUM bank
            if use_g1 or use_b1:
                for t in range(NT):
                    q_ln = q_lnb[:, t, :]
                    if use_g1:
                        nc.vector.tensor_mul(q_ln, q_ln, g1_sb)
                    if use_b1:
                        nc.vector.tensor_add(q_ln, q_ln, b1_sb)
            if use_g2 or use_b2:
                for t in range(NT):
                    if use_g2:
                        nc.vector.tensor_mul(k_raw[:, t, :], k_raw[:, t, :],
                                             g2_sb)
                    if use_b2:
                        nc.vector.tensor_add(k_raw[:, t, :], k_raw[:, t, :],
                                             b2_sb)
            for dstT, src_t, idnt in (
                (qTn, (lambda t: q_lnb[:, t, :]),
                 ident_r if ln_ident1 else ident),
                (kTn, (lambda t: k_raw[:, t, :]), ident),
            ):
                for g0 in range(0, NT, 4):
                    p4 = ps_tr.tile([128, 512],
                                    F32R if (dstT is qTn and ln_ident1)
                                    else F32, tag="pst", name="p4")
                    for k_ in range(4):
                        nc.tensor.matmul(
                            p4[0:64, 128 * k_:128 * k_ + 128],
                            src_t(g0 + k_), idnt, is_transpose=True,
                            start=(k_ == 0), stop=(k_ == 3))
                    nc.vector.tensor_copy(
                        dstT[0:64, g0 * 128:(g0 + 4) * 128], p4[0:64, :])
            nc.sync.dma_start(
                out=ko.ap()[:, hl, :].rearrange("(t p) h -> p t h", p=128),
                in_=k_raw)

        def emit_attn_j(hl, j):
            col = plan[j]
            if not col:
                return
            hs = hstate[hl]
            pr, sub = hl // 2, hl % 2
            qTn = qTn_bufs[hl % 2]
            kTn = kTn_bufs[hl % 2]
            v_aug = hs["v_aug"]
            jsl = slice(j * 512, (j + 1) * 512)
            zt = ps_big.tile([128, 512], F32, tag="big", name="zt")
            for idx, (i, kind, midx) in enumerate(col):
                st_ps = ps_big.tile([128, 512], F32, tag="big", name="st_ps")
                nc.tensor.matmul(
                    st_ps, kTn[:, i * 128:(i + 1) * 128], qTn[:, jsl],
                    start=True, stop=True)
                p = ppool.tile([128, 512], F32R, tag="p", name="p")
                if kind == 0:
                    nc.scalar.activation(p, st_ps,
                                         mybir.ActivationFunctionType.Exp)
                elif kind == 1:
                    nc.scalar.activation(p, st_ps,
                                         mybir.ActivationFunctionType.Exp)
                    # zero strictly-above-diagonal (keep f - p - 128*oi >= 0)
                    nc.gpsimd.affine_select(
                        out=p, in_=p,
                        compare_op=mybir.AluOpType.is_ge, fill=0.0,
                        base=-128 * midx, channel_multiplier=-1,
                        pattern=[[1, 512]])
                else:
                    sm = smt.tile([128, 512], F32, tag="sm", name="sm")
                    nc.vector.tensor_add(sm, st_ps, mk_sb[:, midx, :])
                    nc.scalar.activation(p, sm,
                                         mybir.ActivationFunctionType.Exp)
                nc.tensor.matmul(
                    zt[0:H + 1, :], v_aug[:, i, :], p,
                    start=(idx == 0), stop=(idx == len(col) - 1))
            rd = rd_bufs[(hl * NJ + j) % 2]
            nc.vector.reciprocal(rd[0:1, :], zt[H:H + 1, :])
            bc_sb = bcp.tile([64, 512], F32, tag="bc_sb")
            nc.gpsimd.partition_broadcast(bc_sb, rd[0:1, :])
            nc.vector.tensor_tensor(
                out=zTn_j[j][sub * 64:(sub + 1) * 64, pr, :],
                in0=zt[0:64, :], in1=bc_sb, op=mybir.AluOpType.mult)

        def emit_out_j(j):
            for t in range(4 * j, 4 * j + 4):
                off = (t % 4) * 128
                for dh in range(2):
                    ot = ps_big.tile([128, 512], F32, tag="big", name="ot")
                    for pr in range(2):
                        nc.tensor.matmul(
                            ot, zTn_j[j][:, pr, off:off + 128],
                            wo_sb[:, pr, dh * 512:(dh + 1) * 512],
                            start=(pr == 0), stop=(pr == 1))
                    o_sb = osb.tile([128, 512], F32, tag="o_sb")
                    nc.scalar.copy(o_sb, ot)
                    nc.sync.dma_start(
                        out=outp[t * 128:(t + 1) * 128,
                                 dh * 512:(dh + 1) * 512],
                        in_=o_sb)

        # ---- phase PROJ: weights-stationary f32r, x streamed in 256-wide
        # half-blocks; head-0 raw-backs interleaved per completed j-block ----
        with tc.tile_pool(name="xs", bufs=2) as xs:
            for jh in range(2 * NJ):
                j, half = jh // 2, jh % 2
                csl = slice(j * 512 + half * 256, j * 512 + half * 256 + 256)
                esl = slice(half * 256, half * 256 + 256)
                xq_h = xs.tile([128, NCH, 256], F32R, tag="xq_h",
                               name=f"xq{jh}")
                xk_h = xs.tile([128, NCH, 256], F32R, tag="xk_h",
                               name=f"xk{jh}")
                nc.sync.dma_start(
                    out=xq_h,
                    in_=xqt[:, csl].rearrange("(c p) f -> p c f", p=128))
                nc.sync.dma_start(
                    out=xk_h,
                    in_=xkt[:, csl].rearrange("(c p) f -> p c f", p=128))
                for pr in range(2):
                    ps = ps_big.tile([128, 256], F32, tag="big", name="psq")
                    for c in range(NCH):
                        nc.tensor.matmul(ps, qw2_sb[:, pr, c, :],
                                         xq_h[:, c, :],
                                         start=(c == 0), stop=(c == NCH - 1))
                    nc.vector.tensor_copy(qTraw_j[j][:, pr, esl], ps)
                for n in range(HPC):
                    ps = ps_big.tile([128, 256], F32, tag="big", name="pskv")
                    for c in range(NCH):
                        nc.tensor.matmul(ps, kvw_sb[:, n, c, :],
                                         xk_h[:, c, :],
                                         start=(c == 0), stop=(c == NCH - 1))
                    nc.vector.tensor_copy(kvTraw_j[j][:, n, esl], ps)
                if half == 1:
                    if use_bias:
                        for pr in range(2):
                            nc.vector.tensor_scalar_add(
                                out=qTraw_j[j][:, pr, :],
                                in0=qTraw_j[j][:, pr, :],
                                scalar1=bqc_sb[:, pr:pr + 1])
                        for n in range(HPC):
                            nc.vector.tensor_scalar_add(
                                out=kvTraw_j[j][:, n, :],
                                in0=kvTraw_j[j][:, n, :],
                                scalar1=bkvc_sb[:, n:n + 1])
                    emit_raw(0, range(4 * j, 4 * j + 4))

        # ---- software-pipelined head phases ----
        emit_saf(0)
        for hl in range(HPC):
            for j in range(NJ):
                emit_attn_j(hl, j)
                if hl + 1 < HPC:
                    emit_raw(hl + 1, range(4 * j, 4 * j + 4))
                else:
                    emit_out_j(j)
            if hl + 1 < HPC:
                emit_saf(hl + 1)

    nc.compile()
    return nc


def _make_plan(mask):
    """Classify [sk_chunk=128 x sq_block=512] tiles of the transposed mask."""
    mask = np.asarray(mask, dtype=bool)
    causal = np.array_equal(mask, np.triu(np.ones((SQ, SK), dtype=bool), k=1))
    plan, mtiles = [], []
    if causal:
        for j in range(NJ):
            col = []
            for i in range(NI):
                if i < 4 * j:
                    col.append((i, 0, 0))
                elif i <= 4 * j + 3:
                    col.append((i, 1, i - 4 * j))
            plan.append(col)
        return plan, mtiles
    mt = mask.T  # [sk, sq], True = masked
    for j in range(NJ):
        col = []
        for i in range(NI):
            sub = mt[i * 128:(i + 1) * 128, j * 512:(j + 1) * 512]
            if sub.all():
                continue
            if not sub.any():
                col.append((i, 0, 0))
            else:
                mtiles.append(np.where(sub, np.float32(NEG), np.float32(0.0)))
                col.append((i, 2, len(mtiles) - 1))
        plan.append(col)
    return plan, mtiles


def _get_program(plan, n_mtiles, use_bias, use_g1, use_b1, use_g2, use_b2):
    key = hashlib.sha256(
        repr((plan, n_mtiles, use_bias, use_g1, use_b1, use_g2,
              use_b2)).encode()).hexdigest()
    if key not in _PROGRAM_CACHE:
        _PROGRAM_CACHE[key] = _build_program(
            plan, n_mtiles, use_bias, use_g1, use_b1, use_g2, use_b2)
    return _PROGRAM_CACHE[key]


def kernel(x_q, x_kv, mask, W_Q, W_K, W_V, W_O, b_Q, b_K, b_V, b_O,
           ln1_g, ln1_b, ln2_g, ln2_b):
    x_q = np.asarray(x_q, dtype=np.float32)
    x_kv = np.asarray(x_kv, dtype=np.float32)
    W_Q = np.asarray(W_Q, dtype=np.float32)
    W_K = np.asarray(W_K, dtype=np.float32)
    W_V = np.asarray(W_V, dtype=np.float32)
    W_O = np.asarray(W_O, dtype=np.float32)
    b_Q = np.asarray(b_Q, dtype=np.float32)
    b_K = np.asarray(b_K, dtype=np.float32)
    b_V = np.asarray(b_V, dtype=np.float32)
    b_O = np.asarray(b_O, dtype=np.float32)
    ln1_g = np.asarray(ln1_g, dtype=np.float32)
    ln1_b = np.asarray(ln1_b, dtype=np.float32)
    ln2_g = np.asarray(ln2_g, dtype=np.float32)
    ln2_b = np.asarray(ln2_b, dtype=np.float32)

    plan, mtiles = _make_plan(mask)
    n_mtiles = len(mtiles)
    use_bias = bool(np.any(b_Q) or np.any(b_K) or np.any(b_V))
    use_g1 = not np.all(ln1_g == 1.0)
    use_b1 = bool(np.any(ln1_b))
    use_g2 = not np.all(ln2_g == 1.0)
    use_b2 = bool(np.any(ln2_b))

    nc = _get_program(tuple(map(tuple, plan)), n_mtiles,
                      use_bias, use_g1, use_b1, use_g2, use_b2)

    bf = ml_dtypes.bfloat16
    xqt_b = [np.ascontiguousarray(x_q[b].T) for b in range(B)]
    xkt_b = [np.ascontiguousarray(x_kv[b].T) for b in range(B)]
    if n_mtiles:
        maskt_arr = np.ascontiguousarray(np.stack(mtiles))

    in_maps = []
    for core in range(NCORES):
        b = core // CPB
        n0 = (core % CPB) * HPC
        qw2_arr = np.empty((2, D, 128), dtype=np.float32)
        kvw_arr = np.empty((HPC, D, 128), dtype=np.float32)
        wo2_arr = np.zeros((2, 128, D), dtype=np.float32)
        for pr in range(2):
            qw2_arr[pr, :, 0:64] = W_Q[n0 + 2 * pr]
            qw2_arr[pr, :, 64:128] = W_Q[n0 + 2 * pr + 1]
            wo2_arr[pr, 0:64, :] = W_O[n0 + 2 * pr]
            wo2_arr[pr, 64:128, :] = W_O[n0 + 2 * pr + 1]
        for n in range(HPC):
            kvw_arr[n, :, 0:64] = W_K[n0 + n]
            kvw_arr[n, :, 64:128] = W_V[n0 + n]
        m = {
            "xqt": xqt_b[b],
            "xkt": xkt_b[b],
            "qw2": qw2_arr,
            "kvw": kvw_arr,
            "wo2": wo2_arr.astype(bf),
        }
        if use_bias:
            bqc_arr = np.empty((128, 2), dtype=np.float32)
            bkvc_arr = np.empty((128, HPC), dtype=np.float32)
            for pr in range(2):
                bqc_arr[0:64, pr] = b_Q[n0 + 2 * pr]
                bqc_arr[64:128, pr] = b_Q[n0 + 2 * pr + 1]
            for n in range(HPC):
                bkvc_arr[0:64, n] = b_K[n0 + n]
                bkvc_arr[64:128, n] = b_V[n0 + n]
            m["bqc"] = bqc_arr
            m["bkvc"] = bkvc_arr
        if use_g1:
            m["g1"] = ln1_g
        if use_b1:
            m["b1"] = ln1_b
        if use_g2:
            m["g2"] = ln2_g
        if use_b2:
            m["b2"] = ln2_b
        if n_mtiles:
            m["maskt"] = maskt_arr
        in_maps.append(m)

    res = bass_utils.run_bass_kernel_spmd(nc, in_maps,
                                          core_ids=list(range(NCORES)))

    out = np.zeros((B, SQ, D), dtype=np.float32)
    k_full = np.empty((B, SQ, N, H), dtype=np.float32)
    v_full = np.empty((B, SQ, N, H), dtype=np.float32)
    for core in range(NCORES):
        b = core // CPB
        n0 = (core % CPB) * HPC
        r = res.results[core]
        out[b] += r["outp"]
        k_full[b][:, n0:n0 + HPC, :] = r["ko"]
        v_full[b][:, n0:n0 + HPC, :] = r["vo"]
    out += b_O
    return out, k_full, v_full


# revision 33
# speedup vs baseline: 1.1900x; 1.0792x over previous
"""Trainium2 Bass kernel for nn_AttentionEinOps (B=2, S=2048, D=1024, N=16, H=64).

Sharding: batch x head-block. Core c handles batch b = c // 4 and heads
[4*(c%4), 4*(c%4)+4).  Each core computes q/k/v projections for its 4 heads,
LayerNorm on q and k, causal (or general-masked) attention, and a partial
output projection (sum over its 4 heads).  The host sums the 4 partial outputs
per batch and concatenates the per-core k/v head slices.

Numerics: all attention-path matmuls use float32r (tf32-class, ~1.6e-4 rel
error, full PE rate at N=512).  Projections are computed transposed
(weights-stationary, lhsT = packed weight pairs so M=128) from full-fp32
inputs, transposed back to [s, h] for LayerNorm (exact fp32 stats via
bn_stats), then re-transposed to [h, s] for attention.  Scores are computed
transposed (S^T[sk, sq]) so the softmax denominator falls out of a
ones-augmented v column in the z^T accumulation; no max-subtraction is needed
because |S| <= 64 after LayerNorm (Cauchy-Schwarz).  Output projection runs
in bf16 (negligible error) accumulating 4 heads per PSUM tile.
"""

import hashlib
from contextlib import ExitStack

import numpy as np
import ml_dtypes

import concourse.bass as bass
import concourse.tile as tile
from concourse import bacc, mybir
from concourse import bass_utils
from concourse.masks import make_identity

B, SQ, SK, D, N, H = 2, 2048, 2048, 1024, 16, 64
EPS = 1e-5
NEG = -1e30
NCORES = 8
CPB = NCORES // B      # cores per batch (4)
HPC = N // CPB         # heads per core (4)
NT = SQ // 128         # 16 row tiles
NCH = D // 128         # 8 contraction chunks
NJ = SQ // 512         # 4 sq column blocks
NI = SK // 128         # 16 sk chunks

BF16 = mybir.dt.bfloat16
F32 = mybir.dt.float32
F32R = mybir.dt.float32r
I32 = mybir.dt.int32

_PROGRAM_CACHE: dict = {}


def _build_program(plan, n_mtiles, use_bias, use_g1, use_b1, use_g2, use_b2):
    """plan: per j-block list of (i, kind, idx); kind 0=free, 1=causal diag
    (idx = i-4j in 0..3), 2=general mask tile (idx into maskt input)."""
    nc = bacc.Bacc("TRN2", target_bir_lowering=False, debug=False,
                   num_devices=NCORES)

    xqt = nc.dram_tensor("xqt", [D, SQ], F32R, kind="ExternalInput")
    xkt = nc.dram_tensor("xkt", [D, SK], F32R, kind="ExternalInput")
    qw2 = nc.dram_tensor("qw2", [2, D, 128], F32R, kind="ExternalInput")
    kvw = nc.dram_tensor("kvw", [HPC, D, 128], F32R, kind="ExternalInput")
    wo2 = nc.dram_tensor("wo2", [2, 128, D], BF16, kind="ExternalInput")
    if use_bias:
        bqc = nc.dram_tensor("bqc", [128, 2], F32, kind="ExternalInput")
        bkvc = nc.dram_tensor("bkvc", [128, HPC], F32, kind="ExternalInput")
    if use_g1:
        g1 = nc.dram_tensor("g1", [H], F32, kind="ExternalInput")
    if use_b1:
        b1 = nc.dram_tensor("b1", [H], F32, kind="ExternalInput")
    if use_g2:
        g2 = nc.dram_tensor("g2", [H], F32, kind="ExternalInput")
    if use_b2:
        b2 = nc.dram_tensor("b2", [H], F32, kind="ExternalInput")
    if n_mtiles:
        maskt = nc.dram_tensor("maskt", [n_mtiles, 128, 512], F32,
                               kind="ExternalInput")

    outp = nc.dram_tensor("outp", [SQ, D], F32, kind="ExternalOutput")
    ko = nc.dram_tensor("ko", [SQ, HPC, H], F32, kind="ExternalOutput")
    vo = nc.dram_tensor("vo", [SQ, HPC, H], F32, kind="ExternalOutput")

    any_causal = any(kind == 1 for col in plan for (_, kind, _) in col)
    ln_ident1 = not (use_g1 or use_b1)

    with tile.TileContext(nc) as tc, ExitStack() as ctx:
        c1 = ctx.enter_context(tc.tile_pool(name="c1", bufs=1))

        # ---- phase-proj constants ----
        qw2_sb = c1.tile([128, 2, NCH, 128], F32R, tag="qw2")
        kvw_sb = c1.tile([128, HPC, NCH, 128], F32R, tag="kvw")
        nc.sync.dma_start(out=qw2_sb,
                          in_=qw2.ap().rearrange("q (c p) m -> p q c m", p=128))
        qTraw_j = [c1.tile([128, 2, 512], F32R, tag=f"qTraw{j}",
                           name=f"qTraw{j}") for j in range(NJ)]
        kvTraw_j = [c1.tile([128, HPC, 512], F32R, tag=f"kvTraw{j}",
                            name=f"kvTraw{j}") for j in range(NJ)]
        scr = c1.tile([128, 512], F32, tag="scr")  # constant-fill staging
        nc.vector.memset(scr, 0.0)
        ident = c1.tile([128, 128], F32, tag="ident")
        make_identity(nc, ident)
        ident_r = c1.tile([128, 128], F32R, tag="ident_r")
        nc.vector.tensor_copy(ident_r, ident)
        eps_sb = c1.tile([128, 1], F32, tag="eps")
        nc.vector.memset(eps_sb, EPS)
        onesf = c1.tile([128, H], F32, tag="onesf")
        nc.vector.memset(onesf, 1.0)
        if use_bias:
            bqc_sb = c1.tile([128, 2], F32, tag="bqc")
            bkvc_sb = c1.tile([128, HPC], F32, tag="bkvc")
            nc.sync.dma_start(out=bqc_sb, in_=bqc[:])
            nc.sync.dma_start(out=bkvc_sb, in_=bkvc[:])
        if use_g1:
            g1_sb = c1.tile([128, H], F32, tag="g1")
            nc.sync.dma_start(out=g1_sb, in_=g1.ap().to_broadcast([128, H]))
        if use_b1:
            b1_sb = c1.tile([128, H], F32, tag="b1")
            nc.sync.dma_start(out=b1_sb, in_=b1.ap().to_broadcast([128, H]))
        if use_g2:
            g2_sb = c1.tile([128, H], F32, tag="g2")
            nc.sync.dma_start(out=g2_sb, in_=g2.ap().to_broadcast([128, H]))
        if use_b2:
            b2_sb = c1.tile([128, H], F32, tag="b2")
            nc.sync.dma_start(out=b2_sb, in_=b2.ap().to_broadcast([128, H]))

        # ---- attention-phase pools (before proj so emission can interleave) ----
        c2 = ctx.enter_context(tc.tile_pool(name="c2", bufs=1))
        raws = ctx.enter_context(tc.tile_pool(name="raws", bufs=2))
        stats = ctx.enter_context(tc.tile_pool(name="stats", bufs=2))
        lnt = ctx.enter_context(tc.tile_pool(name="lnt", bufs=2))
        ppool = ctx.enter_context(tc.tile_pool(name="ppool", bufs=3))
        smt = ctx.enter_context(tc.tile_pool(name="smt", bufs=3))
        bcp = ctx.enter_context(tc.tile_pool(name="bcp", bufs=2))
        osb = ctx.enter_context(tc.tile_pool(name="osb", bufs=2))
        ps_tr = ctx.enter_context(
            tc.tile_pool(name="ps_tr", bufs=3, space="PSUM"))
        ps_big = ctx.enter_context(
            tc.tile_pool(name="ps_big", bufs=5, space="PSUM"))

        wo_sb = c2.tile([128, 2, D], BF16, tag="wo")
        qTn_bufs = [c2.tile([128, SQ], F32R, tag=f"qTn{i}", name=f"qTn{i}")
                    for i in range(2)]
        kTn_bufs = [c2.tile([128, SK], F32R, tag=f"kTn{i}", name=f"kTn{i}")
                    for i in range(2)]
        for t_ in qTn_bufs + kTn_bufs:
            for qq in range(4):
                nc.vector.tensor_copy(t_[64:128, qq * 512:(qq + 1) * 512],
                                      scr[64:128, 0:512])
        rd_bufs = [c2.tile([1, 512], F32, tag=f"rd{i}", name=f"rd{i}")
                   for i in range(2)]
        magic_sb = c2.tile([128, NT], I32, tag="magic")
        nc.vector.memset(magic_sb, 0x5f3759df)
        zTn_j = [c2.tile([128, 2, 512], BF16, tag=f"zTn{j}",
                         name=f"zTn{j}") for j in range(NJ)]
        for t_ in zTn_j:
            nc.gpsimd.memset(t_, 0.0)
        if n_mtiles:
            mk_sb = c2.tile([128, n_mtiles, 512], F32, tag="mk")

        hstate = {}

        def emit_alloc(hl):
            q_raw = raws.tile([128, NT, H], F32R, tag="q_raw",
                              name=f"q_raw{hl}")
            k_raw = raws.tile([128, NT, H], F32R, tag="k_raw",
                              name=f"k_raw{hl}")
            v_aug = raws.tile([128, NT, H + 1], F32R, tag="v_aug",
                              name=f"v_aug{hl}")
            nc.vector.tensor_copy(v_aug[:, :, H:H + 1], onesf[:, 0:NT])
            hstate[hl] = dict(
                q_raw=q_raw, k_raw=k_raw, v_aug=v_aug,
                st6_q=stats.tile([128, NT, 6], F32, tag="st6_q",
                                 name=f"st6_q{hl}"),
                st6_k=stats.tile([128, NT, 6], F32, tag="st6_k",
                                 name=f"st6_k{hl}"),
                mn_q=stats.tile([128, NT], F32, tag="mn_q", name=f"mn_q{hl}"),
                mn_k=stats.tile([128, NT], F32, tag="mn_k", name=f"mn_k{hl}"),
                sd_q=stats.tile([128, NT], F32, tag="sd_q", name=f"sd_q{hl}"),
                sd_k=stats.tile([128, NT], F32, tag="sd_k", name=f"sd_k{hl}"),
                tmp_s=stats.tile([128, NT], F32, tag="tmp_s",
                                 name=f"tmp_s{hl}"),
                vv=stats.tile([128, NT], F32, tag="vv", name=f"vv{hl}"),
                hh=stats.tile([128, NT], F32, tag="hh", name=f"hh{hl}"),
                yy=stats.tile([128, NT], F32, tag="yy", name=f"yy{hl}"),
                t1=stats.tile([128, NT], F32, tag="t1", name=f"t1{hl}"),
            )

        def emit_raw(hl, ts):
            """Transpose raw projections of head hl back to [s, h]."""
            if hl not in hstate:
                emit_alloc(hl)
            hs = hstate[hl]
            pr, sub = hl // 2, hl % 2
            ts = list(ts)
            idq = ident_r[sub * 64:(sub + 1) * 64, sub * 64:(sub + 1) * 64]
            for dst, idnt, srcf in (
                ("q_raw", idq,
                 lambda jb, osl: qTraw_j[jb][sub * 64:(sub + 1) * 64, pr, osl]),
                ("k_raw", ident_r[0:64, 0:64],
                 lambda jb, osl: kvTraw_j[jb][0:64, hl, osl]),
                ("v_aug", ident_r[64:128, 64:128],
                 lambda jb, osl: kvTraw_j[jb][64:128, hl, osl]),
            ):
                for g0 in range(0, len(ts), 8):
                    grp = ts[g0:g0 + 8]
                    p8 = ps_tr.tile([128, 512], F32R, tag="pst", name="p8")
                    for k_, t in enumerate(grp):
                        jb, off = t // 4, (t % 4) * 128
                        osl = slice(off, off + 128)
                        nc.tensor.matmul(
                            p8[:, 64 * k_:64 * k_ + 64], srcf(jb, osl), idnt,
                            is_transpose=True, start=(k_ == 0),
                            stop=(k_ == len(grp) - 1))
                    if dst == "v_aug":
                        nc.scalar.copy(
                            hs[dst][:, grp[0]:grp[0] + len(grp), 0:H],
                            p8[:, 0:64 * len(grp)])
                    else:
                        nc.vector.tensor_copy(
                            hs[dst][:, grp[0]:grp[0] + len(grp), :],
                            p8[:, 0:64 * len(grp)])

        def emit_saf(hl):
            """LayerNorm stats + apply + forward transposes + k/v DMAs."""
            hs = hstate[hl]
            pr, sub = hl // 2, hl % 2
            qTn = qTn_bufs[hl % 2]
            kTn = kTn_bufs[hl % 2]
            q_raw, k_raw, v_aug = hs["q_raw"], hs["k_raw"], hs["v_aug"]
            nc.sync.dma_start(
                out=vo.ap()[:, hl, :].rearrange("(t p) h -> p t h", p=128),
                in_=v_aug[:, :, 0:H].bitcast(F32))
            # batched stats; merge bn_stats even/odd halves via Chan:
            # var = (M2e + M2o + 16*(me-mo)^2) / 64;  mean = (me + mo) / 2
            for which in ("q", "k"):
                st6 = hs[f"st6_{which}"]
                mn = hs[f"mn_{which}"]
                sd = hs[f"sd_{which}"]
                tmp_s = hs["tmp_s"]
                src = (q_raw if which == "q" else k_raw).bitcast(F32)
                for t in range(NT):
                    nc.vector.bn_stats(st6[:, t, :], src[:, t, :])
                me, mo = st6[:, :, 1], st6[:, :, 4]
                m2e, m2o = st6[:, :, 2], st6[:, :, 5]
                nc.vector.tensor_tensor(out=mn, in0=me, in1=mo,
                                        op=mybir.AluOpType.add)
                nc.vector.tensor_scalar_mul(out=mn, in0=mn, scalar1=0.5)
                nc.vector.tensor_tensor(out=sd, in0=me, in1=mo,
                                        op=mybir.AluOpType.subtract)
                nc.vector.scalar_tensor_tensor(
                    out=sd, in0=sd, scalar=16.0, in1=sd,
                    op0=mybir.AluOpType.mult, op1=mybir.AluOpType.mult)
                nc.vector.tensor_tensor(out=tmp_s, in0=m2e, in1=m2o,
                                        op=mybir.AluOpType.add)
                nc.vector.tensor_tensor(out=sd, in0=sd, in1=tmp_s,
                                        op=mybir.AluOpType.add)
                # rstd = 1/sqrt(sd/64 + eps) via quake + 2 Newton iters (DVE)
                vv, hh, yy, t1 = hs["vv"], hs["hh"], hs["yy"], hs["t1"]
                nc.vector.tensor_scalar(
                    out=vv, in0=sd, scalar1=1.0 / 64.0, scalar2=EPS,
                    op0=mybir.AluOpType.mult, op1=mybir.AluOpType.add)
                nc.vector.tensor_scalar_mul(out=hh, in0=vv, scalar1=0.5)
                nc.vector.tensor_scalar(
                    out=yy.bitcast(I32), in0=vv.bitcast(I32), scalar1=1,
                    scalar2=None, op0=mybir.AluOpType.arith_shift_right)
                nc.vector.tensor_tensor(
                    out=yy.bitcast(I32), in0=magic_sb, in1=yy.bitcast(I32),
                    op=mybir.AluOpType.subtract)
                for _ in range(3):
                    nc.vector.tensor_tensor(out=t1, in0=yy, in1=yy,
                                            op=mybir.AluOpType.mult)
                    nc.vector.tensor_tensor(out=t1, in0=t1, in1=yy,
                                            op=mybir.AluOpType.mult)
                    nc.vector.tensor_tensor(out=t1, in0=t1, in1=hh,
                                            op=mybir.AluOpType.mult)
                    nc.vector.scalar_tensor_tensor(
                        out=yy, in0=yy, scalar=1.5, in1=t1,
                        op0=mybir.AluOpType.mult,
                        op1=mybir.AluOpType.subtract)
                nc.vector.tensor_copy(sd, yy)
            # batched LN apply in place (broadcast mean/rstd along h)
            nc.vector.tensor_tensor(
                out=q_raw, in0=q_raw.bitcast(F32),
                in1=hs["mn_q"].unsqueeze(2).broadcast_to([128, NT, H]),
                op=mybir.AluOpType.subtract)
            nc.vector.tensor_tensor(
                out=q_raw, in0=q_raw.bitcast(F32),
                in1=hs["sd_q"].unsqueeze(2).broadcast_to([128, NT, H]),
                op=mybir.AluOpType.mult)
            nc.vector.tensor_tensor(
                out=k_raw, in0=k_raw.bitcast(F32),
                in1=hs["mn_k"].unsqueeze(2).broadcast_to([128, NT, H]),
                op=mybir.AluOpType.subtract)
            nc.vector.tensor_tensor(
                out=k_raw, in0=k_raw.bitcast(F32),
                in1=hs["sd_k"].unsqueeze(2).broadcast_to([128, NT, H]),
                op=mybir.AluOpType.mult)
            # forward transposes to [h, s], 4 per PSUM bank
            if use_g1 or use_b1:
                for t in range(NT):
                    q_ln = q_raw[:, t, :]
                    if use_g1:
                        nc.vector.tensor_mul(q_ln, q_ln.bitcast(F32), g1_sb)
                    if use_b1:
                        nc.vector.tensor_add(q_ln, q_ln.bitcast(F32), b1_sb)
            if use_g2 or use_b2:
                for t in range(NT):
                    if use_g2:
                        nc.vector.tensor_mul(
                            k_raw[:, t, :], k_raw.bitcast(F32)[:, t, :],
                            g2_sb)
                    if use_b2:
                        nc.vector.tensor_add(
                            k_raw[:, t, :], k_raw.bitcast(F32)[:, t, :],
                            b2_sb)
            for dstT, src_t, idnt in (
                (qTn, (lambda t: q_raw[:, t, :]), ident_r),
                (kTn, (lambda t: k_raw[:, t, :]), ident_r),
            ):
                for g0 in range(0, NT, 4):
                    p4 = ps_tr.tile([128, 512], F32R, tag="pst", name="p4")
                    for k_ in range(4):
                        nc.tensor.matmul(
                            p4[0:64, 128 * k_:128 * k_ + 128],
                            src_t(g0 + k_), idnt, is_transpose=True,
                            start=(k_ == 0), stop=(k_ == 3))
                    nc.vector.tensor_copy(
                        dstT[0:64, g0 * 128:(g0 + 4) * 128], p4[0:64, :])
            nc.sync.dma_start(
                out=ko.ap()[:, hl, :].rearrange("(t p) h -> p t h", p=128),
                in_=k_raw.bitcast(F32))

        def emit_attn_j(hl, j):
            col = plan[j]
            if not col:
                return
            hs = hstate[hl]
            pr, sub = hl // 2, hl % 2
            qTn = qTn_bufs[hl % 2]
            kTn = kTn_bufs[hl % 2]
            v_aug = hs["v_aug"]
            jsl = slice(j * 512, (j + 1) * 512)
            zt = ps_big.tile([128, 512], F32, tag="big", name="zt")
            for idx, (i, kind, midx) in enumerate(col):
                # diagonal tiles: columns < 128*oi are fully masked -> skip
                o = 128 * midx if kind == 1 else 0
                osl_ = slice(o, 512)
                st_ps = ps_big.tile([128, 512], F32, tag="big", name="st_ps")
                nc.tensor.matmul(
                    st_ps[:, osl_], kTn[:, i * 128:(i + 1) * 128],
                    qTn[:, j * 512 + o:(j + 1) * 512],
                    start=True, stop=True)
                p = ppool.tile([128, 512], F32R, tag="p", name="p")
                if kind == 0:
                    nc.scalar.activation(p, st_ps,
                                         mybir.ActivationFunctionType.Exp)
                elif kind == 1:
                    nc.scalar.activation(p[:, osl_], st_ps[:, osl_],
                                         mybir.ActivationFunctionType.Exp)
                    # zero above-diagonal in the partial 128-col block
                    nc.gpsimd.affine_select(
                        out=p[:, o:o + 128], in_=p[:, o:o + 128],
                        compare_op=mybir.AluOpType.is_ge, fill=0.0,
                        base=0, channel_multiplier=-1,
                        pattern=[[1, 128]])
                else:
                    sm = smt.tile([128, 512], F32, tag="sm", name="sm")
                    nc.vector.tensor_add(sm, st_ps, mk_sb[:, midx, :])
                    nc.scalar.activation(p, sm,
                                         mybir.ActivationFunctionType.Exp)
                nc.tensor.matmul(
                    zt[0:H + 1, osl_], v_aug[:, i, :], p[:, osl_],
                    start=(idx == 0), stop=(idx == len(col) - 1))
            rd = rd_bufs[(hl * NJ + j) % 2]
            nc.vector.reciprocal(rd[0:1, :], zt[H:H + 1, :])
            bc_sb = bcp.tile([64, 512], F32, tag="bc_sb")
            nc.gpsimd.partition_broadcast(bc_sb, rd[0:1, :])
            nc.vector.tensor_tensor(
                out=zTn_j[j][sub * 64:(sub + 1) * 64, pr, :],
                in0=zt[0:64, :], in1=bc_sb, op=mybir.AluOpType.mult)

        def emit_out_j(j):
            for t in range(4 * j, 4 * j + 4):
                off = (t % 4) * 128
                for dh in range(2):
                    ot = ps_big.tile([128, 512], F32, tag="big", name="ot")
                    for pr in range(2):
                        nc.tensor.matmul(
                            ot, zTn_j[j][:, pr, off:off + 128],
                            wo_sb[:, pr, dh * 512:(dh + 1) * 512],
                            start=(pr == 0), stop=(pr == 1))
                    o_sb = osb.tile([128, 512], F32, tag="o_sb")
                    nc.scalar.copy(o_sb, ot)
                    nc.sync.dma_start(
                        out=outp[t * 128:(t + 1) * 128,
                                 dh * 512:(dh + 1) * 512],
                        in_=o_sb)

        # ---- phase PROJ: weights-stationary f32r, x streamed in 256-wide
        # half-blocks; head-0 raw-backs interleaved per completed j-block ----
        with tc.tile_pool(name="xs", bufs=2) as xs:
            for jh in range(2 * NJ):
                j, half = jh // 2, jh % 2
                csl = slice(j * 512 + half * 256, j * 512 + half * 256 + 256)
                esl = slice(half * 256, half * 256 + 256)
                xq_h = xs.tile([128, NCH, 256], F32R, tag="xq_h",
                               name=f"xq{jh}")
                xk_h = xs.tile([128, NCH, 256], F32R, tag="xk_h",
                               name=f"xk{jh}")
                nc.sync.dma_start(
                    out=xq_h,
                    in_=xqt[:, csl].rearrange("(c p) f -> p c f", p=128))
                nc.sync.dma_start(
                    out=xk_h,
                    in_=xkt[:, csl].rearrange("(c p) f -> p c f", p=128))
                if jh == 0:
                    nc.sync.dma_start(
                        out=kvw_sb,
                        in_=kvw.ap().rearrange("n (c p) m -> p n c m", p=128))
                for pr in range(2):
                    ps = ps_big.tile([128, 256], F32, tag="big", name="psq")
                    for c in range(NCH):
                        nc.tensor.matmul(ps, qw2_sb[:, pr, c, :],
                                         xq_h[:, c, :],
                                         start=(c == 0), stop=(c == NCH - 1))
                    nc.vector.tensor_copy(qTraw_j[j][:, pr, esl], ps)
                for n in range(HPC):
                    ps = ps_big.tile([128, 256], F32, tag="big", name="pskv")
                    for c in range(NCH):
                        nc.tensor.matmul(ps, kvw_sb[:, n, c, :],
                                         xk_h[:, c, :],
                                         start=(c == 0), stop=(c == NCH - 1))
                    nc.vector.tensor_copy(kvTraw_j[j][:, n, esl], ps)
                if half == 1:
                    if use_bias:
                        for pr in range(2):
                            nc.vector.tensor_scalar_add(
                                out=qTraw_j[j][:, pr, :],
                                in0=qTraw_j[j][:, pr, :],
                                scalar1=bqc_sb[:, pr:pr + 1])
                        for n in range(HPC):
                            nc.vector.tensor_scalar_add(
                                out=kvTraw_j[j][:, n, :],
                                in0=kvTraw_j[j][:, n, :],
                                scalar1=bkvc_sb[:, n:n + 1])
                    emit_raw(0, range(4 * j, 4 * j + 4))

        # ---- software-pipelined head phases (raw-backs run two heads
        # ahead; saf one head ahead; out-proj interleaved with last head) ----
        nc.sync.dma_start(out=wo_sb,
                          in_=wo2.ap().rearrange("q p d -> p q d"))
        if n_mtiles:
            nc.sync.dma_start(out=mk_sb,
                              in_=maskt.ap().rearrange("m p f -> p m f"))
        emit_saf(0)
        for hl in range(HPC):
            for j in range(NJ):
                emit_attn_j(hl, j)
                if hl + 1 < HPC:
                    emit_raw(hl + 1, range(4 * j, 4 * j + 4))
                else:
                    emit_out_j(j)
            if hl + 1 < HPC:
                emit_saf(hl + 1)

    nc.compile()
    return nc


def _make_plan(mask):
    """Classify [sk_chunk=128 x sq_block=512] tiles of the transposed mask."""
    mask = np.asarray(mask, dtype=bool)
    causal = np.array_equal(mask, np.triu(np.ones((SQ, SK), dtype=bool), k=1))
    plan, mtiles = [], []
    if causal:
        for j in range(NJ):
            col = []
            for i in range(NI):
                if i < 4 * j:
                    col.append((i, 0, 0))
                elif i <= 4 * j + 3:
                    col.append((i, 1, i - 4 * j))
            plan.append(col)
        return plan, mtiles
    mt = mask.T  # [sk, sq], True = masked
    for j in range(NJ):
        col = []
        for i in range(NI):
            sub = mt[i * 128:(i + 1) * 128, j * 512:(j + 1) * 512]
            if sub.all():
                continue
            if not sub.any():
                col.append((i, 0, 0))
            else:
                mtiles.append(np.where(sub, np.float32(NEG), np.float32(0.0)))
                col.append((i, 2, len(mtiles) - 1))
        plan.append(col)
    return plan, mtiles


def _get_program(plan, n_mtiles, use_bias, use_g1, use_b1, use_g2, use_b2):
    key = hashlib.sha256(
        repr((plan, n_mtiles, use_bias, use_g1, use_b1, use_g2,
              use_b2)).encode()).hexdigest()
    if key not in _PROGRAM_CACHE:
        _PROGRAM_CACHE[key] = _build_program(
            plan, n_mtiles, use_bias, use_g1, use_b1, use_g2, use_b2)
    return _PROGRAM_CACHE[key]


def kernel(x_q, x_kv, mask, W_Q, W_K, W_V, W_O, b_Q, b_K, b_V, b_O,
           ln1_g, ln1_b, ln2_g, ln2_b):
    x_q = np.asarray(x_q, dtype=np.float32)
    x_kv = np.asarray(x_kv, dtype=np.float32)
    W_Q = np.asarray(W_Q, dtype=np.float32)
    W_K = np.asarray(W_K, dtype=np.float32)
    W_V = np.asarray(W_V, dtype=np.float32)
    W_O = np.asarray(W_O, dtype=np.float32)
    b_Q = np.asarray(b_Q, dtype=np.float32)
    b_K = np.asarray(b_K, dtype=np.float32)
    b_V = np.asarray(b_V, dtype=np.float32)
    b_O = np.asarray(b_O, dtype=np.float32)
    ln1_g = np.asarray(ln1_g, dtype=np.float32)
    ln1_b = np.asarray(ln1_b, dtype=np.float32)
    ln2_g = np.asarray(ln2_g, dtype=np.float32)
    ln2_b = np.asarray(ln2_b, dtype=np.float32)

    plan, mtiles = _make_plan(mask)
    n_mtiles = len(mtiles)
    use_bias = bool(np.any(b_Q) or np.any(b_K) or np.any(b_V))
    use_g1 = not np.all(ln1_g == 1.0)
    use_b1 = bool(np.any(ln1_b))
    use_g2 = not np.all(ln2_g == 1.0)
    use_b2 = bool(np.any(ln2_b))

    nc = _get_program(tuple(map(tuple, plan)), n_mtiles,
                      use_bias, use_g1, use_b1, use_g2, use_b2)

    bf = ml_dtypes.bfloat16
    xqt_b = [np.ascontiguousarray(x_q[b].T) for b in range(B)]
    xkt_b = [np.ascontiguousarray(x_kv[b].T) for b in range(B)]
    if n_mtiles:
        maskt_arr = np.ascontiguousarray(np.stack(mtiles))

    in_maps = []
    for core in range(NCORES):
        b = core // CPB
        n0 = (core % CPB) * HPC
        qw2_arr = np.empty((2, D, 128), dtype=np.float32)
        kvw_arr = np.empty((HPC, D, 128), dtype=np.float32)
        wo2_arr = np.zeros((2, 128, D), dtype=np.float32)
        for pr in range(2):
            qw2_arr[pr, :, 0:64] = W_Q[n0 + 2 * pr]
            qw2_arr[pr, :, 64:128] = W_Q[n0 + 2 * pr + 1]
            wo2_arr[pr, 0:64, :] = W_O[n0 + 2 * pr]
            wo2_arr[pr, 64:128, :] = W_O[n0 + 2 * pr + 1]
        for n in range(HPC):
            kvw_arr[n, :, 0:64] = W_K[n0 + n]
            kvw_arr[n, :, 64:128] = W_V[n0 + n]
        m = {
            "xqt": xqt_b[b],
            "xkt": xkt_b[b],
            "qw2": qw2_arr,
            "kvw": kvw_arr,
            "wo2": wo2_arr.astype(bf),
        }
        if use_bias:
            bqc_arr = np.empty((128, 2), dtype=np.float32)
            bkvc_arr = np.empty((128, HPC), dtype=np.float32)
            for pr in range(2):
                bqc_arr[0:64, pr] = b_Q[n0 + 2 * pr]
                bqc_arr[64:128, pr] = b_Q[n0 + 2 * pr + 1]
            for n in range(HPC):
                bkvc_arr[0:64, n] = b_K[n0 + n]
                bkvc_arr[64:128, n] = b_V[n0 + n]
            m["bqc"] = bqc_arr
            m["bkvc"] = bkvc_arr
        if use_g1:
            m["g1"] = ln1_g
        if use_b1:
            m["b1"] = ln1_b
        if use_g2:
            m["g2"] = ln2_g
        if use_b2:
            m["b2"] = ln2_b
        if n_mtiles:
            m["maskt"] = maskt_arr
        in_maps.append(m)

    res = bass_utils.run_bass_kernel_spmd(nc, in_maps,
                                          core_ids=list(range(NCORES)))

    out = np.zeros((B, SQ, D), dtype=np.float32)
    k_full = np.empty((B, SQ, N, H), dtype=np.float32)
    v_full = np.empty((B, SQ, N, H), dtype=np.float32)
    for core in range(NCORES):
        b = core // CPB
        n0 = (core % CPB) * HPC
        r = res.results[core]
        out[b] += r["outp"]
        k_full[b][:, n0:n0 + HPC, :] = r["ko"]
        v_full[b][:, n0:n0 + HPC, :] = r["vo"]
    out += b_O
    return out, k_full, v_full


# revision 51
# speedup vs baseline: 22877.1468x; 19225.2903x over previous
"""Trainium2 Bass kernel for nn_AttentionEinOps (B=2, S=2048, D=1024, N=16, H=64).

Sharding: batch x head-block. Core c handles batch b = c // 4 and heads
[4*(c%4), 4*(c%4)+4).  Each core computes q/k/v projections for its 4 heads,
LayerNorm on q and k, causal (or general-masked) attention, and a partial
output projection (sum over its 4 heads).  The host sums the 4 partial outputs
per batch and concatenates the per-core k/v head slices.

Numerics: all attention-path matmuls use float32r (tf32-class, ~1.6e-4 rel
error, full PE rate at N>=256).  Projections are computed transposed
(weights-stationary, lhsT = packed weight pairs so M=128) from full-fp32
inputs, transposed back to [s, h] for LayerNorm (fp32 stats via bn_stats +
a DVE-only quake rsqrt to avoid activation-table swaps), then re-transposed
to [h, s] for attention.  Scores are computed transposed (S^T[sk, sq]) so
the softmax denominator falls out of a ones-augmented v column in the z^T
accumulation; no max-subtraction is needed because |S| <= 64 after LayerNorm
(Cauchy-Schwarz).  Diagonal (causal-partial) score tiles skip their fully
masked 128-column blocks and zero the straddling block with a gpsimd
affine_select after exp.  Output projection runs in bf16 accumulating both
heads of a pair per matmul via the zero-padded h rows.

Scheduling: emission order is the per-engine execution order, so phases are
software-pipelined by hand — head-0 raw-back transposes interleave with the
x-streamed projection loop, each head's attention interleaves the next
head's raw-backs and stats/apply/forward-transposes (split in two), and the
output projection interleaves with the last head's attention.  PSUM-bank
batched transposes (8 per bank via start/stop accumulation groups) cut the
eviction instruction count ~6x.  Cost model: ~224 us; measured on silicon
via a 41x-repeated-pipeline NEFF: ~230-270 us per iteration.
"""

import hashlib
from contextlib import ExitStack

import numpy as np
import ml_dtypes

import concourse.bass as bass
import concourse.tile as tile
from concourse import bacc, mybir
from concourse import bass_utils
from concourse.masks import make_identity

B, SQ, SK, D, N, H = 2, 2048, 2048, 1024, 16, 64
EPS = 1e-5
NEG = -1e30
NCORES = 8
CPB = NCORES // B      # cores per batch (4)
HPC = N // CPB         # heads per core (4)
NT = SQ // 128         # 16 row tiles
NCH = D // 128         # 8 contraction chunks
NJ = SQ // 512         # 4 sq column blocks
NI = SK // 128         # 16 sk chunks

BF16 = mybir.dt.bfloat16
F32 = mybir.dt.float32
F32R = mybir.dt.float32r
I32 = mybir.dt.int32

_PROGRAM_CACHE: dict = {}


def _build_program(plan, n_mtiles, use_bias, use_g1, use_b1, use_g2, use_b2,
                   reps=1):
    """plan: per j-block list of (i, kind, idx); kind 0=free, 1=causal diag
    (idx = i-4j in 0..3), 2=general mask tile (idx into maskt input)."""
    nc = bacc.Bacc("TRN2", target_bir_lowering=False, debug=False,
                   num_devices=NCORES)

    xqt = nc.dram_tensor("xqt", [D, SQ], F32R, kind="ExternalInput")
    xkt = nc.dram_tensor("xkt", [D, SK], F32R, kind="ExternalInput")
    qw2 = nc.dram_tensor("qw2", [2, D, 128], F32R, kind="ExternalInput")
    kvw = nc.dram_tensor("kvw", [HPC, D, 128], F32R, kind="ExternalInput")
    wo2 = nc.dram_tensor("wo2", [2, 128, D], BF16, kind="ExternalInput")
    if use_bias:
        bqc = nc.dram_tensor("bqc", [128, 2], F32, kind="ExternalInput")
        bkvc = nc.dram_tensor("bkvc", [128, HPC], F32, kind="ExternalInput")
    if use_g1:
        g1 = nc.dram_tensor("g1", [H], F32, kind="ExternalInput")
    if use_b1:
        b1 = nc.dram_tensor("b1", [H], F32, kind="ExternalInput")
    if use_g2:
        g2 = nc.dram_tensor("g2", [H], F32, kind="ExternalInput")
    if use_b2:
        b2 = nc.dram_tensor("b2", [H], F32, kind="ExternalInput")
    if n_mtiles:
        maskt = nc.dram_tensor("maskt", [n_mtiles, 128, 512], F32,
                               kind="ExternalInput")

    outp = nc.dram_tensor("outp", [SQ, D], F32, kind="ExternalOutput")
    ko = nc.dram_tensor("ko", [SQ, HPC, H], F32, kind="ExternalOutput")
    vo = nc.dram_tensor("vo", [SQ, HPC, H], F32, kind="ExternalOutput")

    any_causal = any(kind == 1 for col in plan for (_, kind, _) in col)
    ln_ident1 = not (use_g1 or use_b1)

    with tile.TileContext(nc) as tc, ExitStack() as ctx:
        c1 = ctx.enter_context(tc.tile_pool(name="c1", bufs=1))

        # ---- phase-proj constants ----
        qw2_sb = c1.tile([128, 2, NCH, 128], F32R, tag="qw2")
        kvw_sb = c1.tile([128, HPC, NCH, 128], F32R, tag="kvw")
        nc.sync.dma_start(out=qw2_sb,
                          in_=qw2.ap().rearrange("q (c p) m -> p q c m", p=128))
        qTraw_j = [c1.tile([128, 2, 512], F32R, tag=f"qTraw{j}",
                           name=f"qTraw{j}") for j in range(NJ)]
        kvTraw_j = [c1.tile([128, HPC, 512], F32R, tag=f"kvTraw{j}",
                            name=f"kvTraw{j}") for j in range(NJ)]
        scr = c1.tile([128, 512], F32, tag="scr")  # constant-fill staging
        nc.vector.memset(scr, 0.0)
        ident = c1.tile([128, 128], F32, tag="ident")
        make_identity(nc, ident)
        ident_r = c1.tile([128, 128], F32R, tag="ident_r")
        nc.vector.tensor_copy(ident_r, ident)
        eps_sb = c1.tile([128, 1], F32, tag="eps")
        nc.vector.memset(eps_sb, EPS)
        onesf = c1.tile([128, H], F32, tag="onesf")
        nc.vector.memset(onesf, 1.0)
        if use_bias:
            bqc_sb = c1.tile([128, 2], F32, tag="bqc")
            bkvc_sb = c1.tile([128, HPC], F32, tag="bkvc")
            nc.sync.dma_start(out=bqc_sb, in_=bqc[:])
            nc.sync.dma_start(out=bkvc_sb, in_=bkvc[:])
        def _bcast128(dram_t):
            a = dram_t.ap()
            return bass.AP(tensor=a.tensor, offset=a.offset,
                           ap=[[0, 128]] + list(a.ap))
        if use_g1:
            g1_sb = c1.tile([128, H], F32, tag="g1")
            nc.sync.dma_start(out=g1_sb, in_=_bcast128(g1))
        if use_b1:
            b1_sb = c1.tile([128, H], F32, tag="b1")
            nc.sync.dma_start(out=b1_sb, in_=_bcast128(b1))
        if use_g2:
            g2_sb = c1.tile([128, H], F32, tag="g2")
            nc.sync.dma_start(out=g2_sb, in_=_bcast128(g2))
        if use_b2:
            b2_sb = c1.tile([128, H], F32, tag="b2")
            nc.sync.dma_start(out=b2_sb, in_=_bcast128(b2))

        # ---- attention-phase pools (before proj so emission can interleave) ----
        c2 = ctx.enter_context(tc.tile_pool(name="c2", bufs=1))
        raws = ctx.enter_context(tc.tile_pool(name="raws", bufs=2))
        stats = ctx.enter_context(tc.tile_pool(name="stats", bufs=2))
        lnt = ctx.enter_context(tc.tile_pool(name="lnt", bufs=2))
        ppool = ctx.enter_context(tc.tile_pool(name="ppool", bufs=4))
        smt = ctx.enter_context(tc.tile_pool(name="smt", bufs=2))
        bcp = ctx.enter_context(tc.tile_pool(name="bcp", bufs=2))
        osb = ctx.enter_context(tc.tile_pool(name="osb", bufs=2))
        ps_tr = ctx.enter_context(
            tc.tile_pool(name="ps_tr", bufs=3, space="PSUM"))
        ps_big = ctx.enter_context(
            tc.tile_pool(name="ps_big", bufs=5, space="PSUM"))

        wo_sb = c2.tile([128, 2, D], BF16, tag="wo")
        qTn_bufs = [c2.tile([128, SQ], F32R, tag=f"qTn{i}", name=f"qTn{i}")
                    for i in range(2)]
        kTn_bufs = [c2.tile([128, SK], F32R, tag=f"kTn{i}", name=f"kTn{i}")
                    for i in range(2)]
        for t_ in qTn_bufs + kTn_bufs:
            for qq in range(4):
                nc.vector.tensor_copy(t_[64:128, qq * 512:(qq + 1) * 512],
                                      scr[64:128, 0:512])
        rd_bufs = [c2.tile([1, 512], F32, tag=f"rd{i}", name=f"rd{i}")
                   for i in range(2)]
        magic_sb = c2.tile([128, NT], I32, tag="magic")
        nc.vector.memset(magic_sb, 0x5f3759df)
        zTn_j = [c2.tile([128, 2, 512], BF16, tag=f"zTn{j}",
                         name=f"zTn{j}") for j in range(NJ)]
        for t_ in zTn_j:
            nc.gpsimd.memset(t_, 0.0)
        preload_mask = 0 < n_mtiles <= 16
        if preload_mask:
            mk_sb = c2.tile([128, n_mtiles, 512], F32, tag="mk")

        hstate = {}

        def emit_alloc(hl):
            q_raw = raws.tile([128, NT, H], F32R, tag="q_raw",
                              name=f"q_raw{hl}")
            k_raw = raws.tile([128, NT, H], F32R, tag="k_raw",
                              name=f"k_raw{hl}")
            v_aug = raws.tile([128, NT, H + 1], F32R, tag="v_aug",
                              name=f"v_aug{hl}")
            nc.vector.tensor_copy(v_aug[:, :, H:H + 1], onesf[:, 0:NT])
            hstate[hl] = dict(
                q_raw=q_raw, k_raw=k_raw, v_aug=v_aug,
                st6_q=stats.tile([128, NT, 6], F32, tag="st6_q",
                                 name=f"st6_q{hl}"),
                st6_k=stats.tile([128, NT, 6], F32, tag="st6_k",
                                 name=f"st6_k{hl}"),
                mn_q=stats.tile([128, NT], F32, tag="mn_q", name=f"mn_q{hl}"),
                mn_k=stats.tile([128, NT], F32, tag="mn_k", name=f"mn_k{hl}"),
                sd_q=stats.tile([128, NT], F32, tag="sd_q", name=f"sd_q{hl}"),
                sd_k=stats.tile([128, NT], F32, tag="sd_k", name=f"sd_k{hl}"),
                tmp_s=stats.tile([128, NT], F32, tag="tmp_s",
                                 name=f"tmp_s{hl}"),
                vv=stats.tile([128, NT], F32, tag="vv", name=f"vv{hl}"),
                hh=stats.tile([128, NT], F32, tag="hh", name=f"hh{hl}"),
                yy=stats.tile([128, NT], F32, tag="yy", name=f"yy{hl}"),
                t1=stats.tile([128, NT], F32, tag="t1", name=f"t1{hl}"),
            )

        def emit_raw(hl, ts):
            """Transpose raw projections of head hl back to [s, h]."""
            if hl not in hstate:
                emit_alloc(hl)
            hs = hstate[hl]
            pr, sub = hl // 2, hl % 2
            ts = list(ts)
            idq = ident_r[sub * 64:(sub + 1) * 64, sub * 64:(sub + 1) * 64]
            for dst, idnt, srcf in (
                ("q_raw", idq,
                 lambda jb, osl: qTraw_j[jb][sub * 64:(sub + 1) * 64, pr, osl]),
                ("k_raw", ident_r[0:64, 0:64],
                 lambda jb, osl: kvTraw_j[jb][0:64, hl, osl]),
                ("v_aug", ident_r[64:128, 64:128],
                 lambda jb, osl: kvTraw_j[jb][64:128, hl, osl]),
            ):
                for g0 in range(0, len(ts), 8):
                    grp = ts[g0:g0 + 8]
                    p8 = ps_tr.tile([128, 512], F32R, tag="pst", name="p8")
                    for k_, t in enumerate(grp):
                        jb, off = t // 4, (t % 4) * 128
                        osl = slice(off, off + 128)
                        nc.tensor.matmul(
                            p8[:, 64 * k_:64 * k_ + 64], srcf(jb, osl), idnt,
                            is_transpose=True, start=(k_ == 0),
                            stop=(k_ == len(grp) - 1))
                    if dst == "v_aug":
                        nc.scalar.copy(
                            hs[dst][:, grp[0]:grp[0] + len(grp), 0:H],
                            p8[:, 0:64 * len(grp)])
                    else:
                        nc.vector.tensor_copy(
                            hs[dst][:, grp[0]:grp[0] + len(grp), :],
                            p8[:, 0:64 * len(grp)])

        def emit_saf(hl, part="all"):
            """LayerNorm stats + apply + forward transposes + k/v DMAs."""
            hs = hstate[hl]
            pr, sub = hl // 2, hl % 2
            qTn = qTn_bufs[hl % 2]
            kTn = kTn_bufs[hl % 2]
            q_raw, k_raw, v_aug = hs["q_raw"], hs["k_raw"], hs["v_aug"]
            if part in ("all", "stats"):
                emit_saf_stats(hl, hs, pr, sub, q_raw, k_raw, v_aug)
            if part in ("all", "fwd"):
                emit_saf_fwd(hl, hs, pr, sub, qTn, kTn, q_raw, k_raw)

        def emit_saf_stats(hl, hs, pr, sub, q_raw, k_raw, v_aug):
            nc.sync.dma_start(
                out=vo.ap()[:, hl, :].rearrange("(t p) h -> p t h", p=128),
                in_=v_aug[:, :, 0:H].bitcast(F32))
            # batched stats; merge bn_stats even/odd halves via Chan:
            # var = (M2e + M2o + 16*(me-mo)^2) / 64;  mean = (me + mo) / 2
            for which in ("q", "k"):
                st6 = hs[f"st6_{which}"]
                mn = hs[f"mn_{which}"]
                sd = hs[f"sd_{which}"]
                tmp_s = hs["tmp_s"]
                src = (q_raw if which == "q" else k_raw).bitcast(F32)
                for t in range(NT):
                    nc.vector.bn_stats(st6[:, t, :], src[:, t, :])
                me, mo = st6[:, :, 1], st6[:, :, 4]
                m2e, m2o = st6[:, :, 2], st6[:, :, 5]
                nc.vector.tensor_tensor(out=mn, in0=me, in1=mo,
                                        op=mybir.AluOpType.add)
                nc.vector.tensor_scalar_mul(out=mn, in0=mn, scalar1=0.5)
                nc.vector.tensor_tensor(out=sd, in0=me, in1=mo,
                                        op=mybir.AluOpType.subtract)
                nc.vector.scalar_tensor_tensor(
                    out=sd, in0=sd, scalar=16.0, in1=sd,
                    op0=mybir.AluOpType.mult, op1=mybir.AluOpType.mult)
                nc.vector.tensor_tensor(out=tmp_s, in0=m2e, in1=m2o,
                                        op=mybir.AluOpType.add)
                nc.vector.tensor_tensor(out=sd, in0=sd, in1=tmp_s,
                                        op=mybir.AluOpType.add)
                # rstd = 1/sqrt(sd/64 + eps) via quake + 2 Newton iters (DVE)
                vv, hh, yy, t1 = hs["vv"], hs["hh"], hs["yy"], hs["t1"]
                nc.vector.tensor_scalar(
                    out=vv, in0=sd, scalar1=1.0 / 64.0, scalar2=EPS,
                    op0=mybir.AluOpType.mult, op1=mybir.AluOpType.add)
                nc.vector.tensor_scalar_mul(out=hh, in0=vv, scalar1=0.5)
                nc.vector.tensor_scalar(
                    out=yy.bitcast(I32), in0=vv.bitcast(I32), scalar1=1,
                    scalar2=None, op0=mybir.AluOpType.arith_shift_right)
                nc.vector.tensor_tensor(
                    out=yy.bitcast(I32), in0=magic_sb, in1=yy.bitcast(I32),
                    op=mybir.AluOpType.subtract)
                for _ in range(3):
                    nc.vector.tensor_tensor(out=t1, in0=yy, in1=yy,
                                            op=mybir.AluOpType.mult)
                    nc.vector.tensor_tensor(out=t1, in0=t1, in1=yy,
                                            op=mybir.AluOpType.mult)
                    nc.vector.tensor_tensor(out=t1, in0=t1, in1=hh,
                                            op=mybir.AluOpType.mult)
                    nc.vector.scalar_tensor_tensor(
                        out=yy, in0=yy, scalar=1.5, in1=t1,
                        op0=mybir.AluOpType.mult,
                        op1=mybir.AluOpType.subtract)
                nc.vector.tensor_copy(sd, yy)
            # LN apply in place, per tile (2x-rate tensor_scalar)
            for t in range(NT):
                nc.vector.tensor_scalar(
                    out=q_raw[:, t, :], in0=q_raw.bitcast(F32)[:, t, :],
                    scalar1=hs["mn_q"][:, t:t + 1],
                    scalar2=hs["sd_q"][:, t:t + 1],
                    op0=mybir.AluOpType.subtract, op1=mybir.AluOpType.mult)
                nc.vector.tensor_scalar(
                    out=k_raw[:, t, :], in0=k_raw.bitcast(F32)[:, t, :],
                    scalar1=hs["mn_k"][:, t:t + 1],
                    scalar2=hs["sd_k"][:, t:t + 1],
                    op0=mybir.AluOpType.subtract, op1=mybir.AluOpType.mult)
        def emit_saf_fwd(hl, hs, pr, sub, qTn, kTn, q_raw, k_raw):
            # forward transposes to [h, s], 4 per PSUM bank
            if use_g1 or use_b1:
                for t in range(NT):
                    q_ln = q_raw[:, t, :]
                    if use_g1:
                        nc.vector.tensor_mul(q_ln, q_ln.bitcast(F32), g1_sb)
                    if use_b1:
                        nc.vector.tensor_add(q_ln, q_ln.bitcast(F32), b1_sb)
            if use_g2 or use_b2:
                for t in range(NT):
                    if use_g2:
                        nc.vector.tensor_mul(
                            k_raw[:, t, :], k_raw.bitcast(F32)[:, t, :],
                            g2_sb)
                    if use_b2:
                        nc.vector.tensor_add(
                            k_raw[:, t, :], k_raw.bitcast(F32)[:, t, :],
                            b2_sb)
            for dstT, src_t, idnt in (
                (qTn, (lambda t: q_raw[:, t, :]), ident_r),
                (kTn, (lambda t: k_raw[:, t, :]), ident_r),
            ):
                for g0 in range(0, NT, 4):
                    p4 = ps_tr.tile([128, 512], F32R, tag="pst", name="p4")
                    for k_ in range(4):
                        nc.tensor.matmul(
                            p4[0:64, 128 * k_:128 * k_ + 128],
                            src_t(g0 + k_), idnt, is_transpose=True,
                            start=(k_ == 0), stop=(k_ == 3))
                    nc.vector.tensor_copy(
                        dstT[0:64, g0 * 128:(g0 + 4) * 128], p4[0:64, :])
            nc.sync.dma_start(
                out=ko.ap()[:, hl, :].rearrange("(t p) h -> p t h", p=128),
                in_=k_raw.bitcast(F32))

        def emit_attn_j(hl, j):
            col = plan[j]
            if not col:
                return
            hs = hstate[hl]
            pr, sub = hl // 2, hl % 2
            qTn = qTn_bufs[hl % 2]
            kTn = kTn_bufs[hl % 2]
            v_aug = hs["v_aug"]
            jsl = slice(j * 512, (j + 1) * 512)
            zt = ps_big.tile([128, 512], F32, tag="big", name="zt")
            for idx, (i, kind, midx) in enumerate(col):
                # diagonal tiles: columns < 128*oi are fully masked -> skip
                o = 128 * midx if kind == 1 else 0
                osl_ = slice(o, 512)
                st_ps = ps_big.tile([128, 512], F32, tag="big", name="st_ps")
                nc.tensor.matmul(
                    st_ps[:, osl_], kTn[:, i * 128:(i + 1) * 128],
                    qTn[:, j * 512 + o:(j + 1) * 512],
                    start=True, stop=True)
                p = ppool.tile([128, 512], F32R, tag="p", name="p")
                if kind == 0:
                    nc.scalar.activation(p, st_ps,
                                         mybir.ActivationFunctionType.Exp)
                elif kind == 1:
                    nc.scalar.activation(p[:, osl_], st_ps[:, osl_],
                                         mybir.ActivationFunctionType.Exp)
                    # zero above-diagonal in the partial 128-col block
                    nc.gpsimd.affine_select(
                        out=p[:, o:o + 128], in_=p[:, o:o + 128],
                        compare_op=mybir.AluOpType.is_ge, fill=0.0,
                        base=0, channel_multiplier=-1,
                        pattern=[[1, 128]])
                else:
                    if preload_mask:
                        mtile = mk_sb[:, midx, :]
                    else:
                        mtile = smt.tile([128, 512], F32, tag="mdma",
                                         name="mdma")
                        nc.sync.dma_start(out=mtile, in_=maskt[midx])
                    sm = smt.tile([128, 512], F32, tag="sm", name="sm")
                    nc.vector.tensor_add(sm, st_ps, mtile)
                    nc.scalar.activation(p, sm,
                                         mybir.ActivationFunctionType.Exp)
                nc.tensor.matmul(
                    zt[0:H + 1, osl_], v_aug[:, i, :], p[:, osl_],
                    start=(idx == 0), stop=(idx == len(col) - 1))
            rd = rd_bufs[(hl * NJ + j) % 2]
            nc.vector.reciprocal(rd[0:1, :], zt[H:H + 1, :])
            bc_sb = bcp.tile([64, 512], F32, tag="bc_sb")
            nc.gpsimd.partition_broadcast(bc_sb, rd[0:1, :])
            nc.vector.tensor_tensor(
                out=zTn_j[j][sub * 64:(sub + 1) * 64, pr, :],
                in0=zt[0:64, :], in1=bc_sb, op=mybir.AluOpType.mult)

        def emit_out_j(j):
            for t in range(4 * j, 4 * j + 4):
                off = (t % 4) * 128
                for dh in range(2):
                    ot = ps_big.tile([128, 512], F32, tag="big", name="ot")
                    for pr in range(2):
                        nc.tensor.matmul(
                            ot, zTn_j[j][:, pr, off:off + 128],
                            wo_sb[:, pr, dh * 512:(dh + 1) * 512],
                            start=(pr == 0), stop=(pr == 1))
                    o_sb = osb.tile([128, 512], F32, tag="o_sb")
                    nc.vector.tensor_copy(o_sb, ot)
                    nc.sync.dma_start(
                        out=outp[t * 128:(t + 1) * 128,
                                 dh * 512:(dh + 1) * 512],
                        in_=o_sb)

        # ---- phase PROJ: weights-stationary f32r, x streamed in 256-wide
        # half-blocks; head-0 raw-backs interleaved per completed j-block ----
        for _rep in range(reps):
          hstate.clear()
          with tc.tile_pool(name="xs", bufs=2) as xs:
            for jh in range(2 * NJ):
                j, half = jh // 2, jh % 2
                csl = slice(j * 512 + half * 256, j * 512 + half * 256 + 256)
                esl = slice(half * 256, half * 256 + 256)
                xq_h = xs.tile([128, NCH, 256], F32R, tag="xq_h",
                               name=f"xq{jh}")
                xk_h = xs.tile([128, NCH, 256], F32R, tag="xk_h",
                               name=f"xk{jh}")
                nc.sync.dma_start(
                    out=xq_h,
                    in_=xqt[:, csl].rearrange("(c p) f -> p c f", p=128))
                nc.sync.dma_start(
                    out=xk_h,
                    in_=xkt[:, csl].rearrange("(c p) f -> p c f", p=128))
                if jh == 0:
                    nc.sync.dma_start(
                        out=kvw_sb,
                        in_=kvw.ap().rearrange("n (c p) m -> p n c m", p=128))
                for pr in range(2):
                    ps = ps_big.tile([128, 256], F32, tag="big", name="psq")
                    for c in range(NCH):
                        nc.tensor.matmul(ps, qw2_sb[:, pr, c, :],
                                         xq_h[:, c, :],
                                         start=(c == 0), stop=(c == NCH - 1))
                    nc.scalar.copy(qTraw_j[j][:, pr, esl], ps)
                for n in range(HPC):
                    ps = ps_big.tile([128, 256], F32, tag="big", name="pskv")
                    for c in range(NCH):
                        nc.tensor.matmul(ps, kvw_sb[:, n, c, :],
                                         xk_h[:, c, :],
                                         start=(c == 0), stop=(c == NCH - 1))
                    nc.scalar.copy(kvTraw_j[j][:, n, esl], ps)
                if half == 1:
                    if use_bias:
                        for pr in range(2):
                            nc.vector.tensor_scalar_add(
                                out=qTraw_j[j][:, pr, :],
                                in0=qTraw_j[j][:, pr, :],
                                scalar1=bqc_sb[:, pr:pr + 1])
                        for n in range(HPC):
                            nc.vector.tensor_scalar_add(
                                out=kvTraw_j[j][:, n, :],
                                in0=kvTraw_j[j][:, n, :],
                                scalar1=bkvc_sb[:, n:n + 1])
                    emit_raw(0, range(4 * j, 4 * j + 4))

          # ---- software-pipelined head phases (raw-backs run two heads
          # ahead; saf one head ahead; out-proj interleaved with last head) ----
          if _rep == 0:
            nc.sync.dma_start(out=wo_sb,
                              in_=wo2.ap().rearrange("q p d -> p q d"))
            if preload_mask:
                nc.sync.dma_start(out=mk_sb,
                                  in_=maskt.ap().rearrange("m p f -> p m f"))
          emit_saf(0)
          for hl in range(HPC):
            last = hl + 1 == HPC
            emit_attn_j(hl, 0)
            if not last:
                emit_raw(hl + 1, range(0, 8))
            emit_attn_j(hl, 1)
            if not last:
                emit_raw(hl + 1, range(8, 16))
            else:
                emit_out_j(0)
            emit_attn_j(hl, 2)
            if not last:
                emit_saf(hl + 1, "stats")
            else:
                emit_out_j(1)
            emit_attn_j(hl, 3)
            if not last:
                emit_saf(hl + 1, "fwd")
            else:
                emit_out_j(2)
                emit_out_j(3)

    nc.compile()
    return nc


def _make_plan(mask):
    """Classify [sk_chunk=128 x sq_block=512] tiles of the transposed mask."""
    mask = np.asarray(mask, dtype=bool)
    causal = np.array_equal(mask, np.triu(np.ones((SQ, SK), dtype=bool), k=1))
    plan, mtiles = [], []
    if causal:
        for j in range(NJ):
            col = []
            for i in range(NI):
                if i < 4 * j:
                    col.append((i, 0, 0))
                elif i <= 4 * j + 3:
                    col.append((i, 1, i - 4 * j))
            plan.append(col)
        return plan, mtiles
    mt = mask.T  # [sk, sq], True = masked
    for j in range(NJ):
        col = []
        for i in range(NI):
            sub = mt[i * 128:(i + 1) * 128, j * 512:(j + 1) * 512]
            if sub.all():
                continue
            if not sub.any():
                col.append((i, 0, 0))
            else:
                mtiles.append(np.where(sub, np.float32(NEG), np.float32(0.0)))
                col.append((i, 2, len(mtiles) - 1))
        plan.append(col)
    return plan, mtiles


def _get_program(plan, n_mtiles, use_bias, use_g1, use_b1, use_g2, use_b2):
    key = hashlib.sha256(
        repr((plan, n_mtiles, use_bias, use_g1, use_b1, use_g2,
              use_b2)).encode()).hexdigest()
    if key not in _PROGRAM_CACHE:
        _PROGRAM_CACHE[key] = _build_program(
            plan, n_mtiles, use_bias, use_g1, use_b1, use_g2, use_b2)
    return _PROGRAM_CACHE[key]


def kernel(x_q, x_kv, mask, W_Q, W_K, W_V, W_O, b_Q, b_K, b_V, b_O,
           ln1_g, ln1_b, ln2_g, ln2_b):
    x_q = np.asarray(x_q, dtype=np.float32)
    x_kv = np.asarray(x_kv, dtype=np.float32)
    W_Q = np.asarray(W_Q, dtype=np.float32)
    W_K = np.asarray(W_K, dtype=np.float32)
    W_V = np.asarray(W_V, dtype=np.float32)
    W_O = np.asarray(W_O, dtype=np.float32)
    b_Q = np.asarray(b_Q, dtype=np.float32)
    b_K = np.asarray(b_K, dtype=np.float32)
    b_V = np.asarray(b_V, dtype=np.float32)
    b_O = np.asarray(b_O, dtype=np.float32)
    ln1_g = np.asarray(ln1_g, dtype=np.float32)
    ln1_b = np.asarray(ln1_b, dtype=np.float32)
    ln2_g = np.asarray(ln2_g, dtype=np.float32)
    ln2_b = np.asarray(ln2_b, dtype=np.float32)

    plan, mtiles = _make_plan(mask)
    n_mtiles = len(mtiles)
    use_bias = bool(np.any(b_Q) or np.any(b_K) or np.any(b_V))
    use_g1 = not np.all(ln1_g == 1.0)
    use_b1 = bool(np.any(ln1_b))
    use_g2 = not np.all(ln2_g == 1.0)
    use_b2 = bool(np.any(ln2_b))

    nc = _get_program(tuple(map(tuple, plan)), n_mtiles,
                      use_bias, use_g1, use_b1, use_g2, use_b2)

    bf = ml_dtypes.bfloat16
    xqt_b = [np.ascontiguousarray(x_q[b].T) for b in range(B)]
    xkt_b = [np.ascontiguousarray(x_kv[b].T) for b in range(B)]
    if n_mtiles:
        maskt_arr = np.ascontiguousarray(np.stack(mtiles))

    in_maps = []
    for core in range(NCORES):
        b = core // CPB
        n0 = (core % CPB) * HPC
        qw2_arr = np.empty((2, D, 128), dtype=np.float32)
        kvw_arr = np.empty((HPC, D, 128), dtype=np.float32)
        wo2_arr = np.zeros((2, 128, D), dtype=np.float32)
        for pr in range(2):
            qw2_arr[pr, :, 0:64] = W_Q[n0 + 2 * pr]
            qw2_arr[pr, :, 64:128] = W_Q[n0 + 2 * pr + 1]
            wo2_arr[pr, 0:64, :] = W_O[n0 + 2 * pr]
            wo2_arr[pr, 64:128, :] = W_O[n0 + 2 * pr + 1]
        for n in range(HPC):
            kvw_arr[n, :, 0:64] = W_K[n0 + n]
            kvw_arr[n, :, 64:128] = W_V[n0 + n]
        m = {
            "xqt": xqt_b[b],
            "xkt": xkt_b[b],
            "qw2": qw2_arr,
            "kvw": kvw_arr,
            "wo2": wo2_arr.astype(bf),
        }
        if use_bias:
            bqc_arr = np.empty((128, 2), dtype=np.float32)
            bkvc_arr = np.empty((128, HPC), dtype=np.float32)
            for pr in range(2):
                bqc_arr[0:64, pr] = b_Q[n0 + 2 * pr]
                bqc_arr[64:128, pr] = b_Q[n0 + 2 * pr + 1]
            for n in range(HPC):
                bkvc_arr[0:64, n] = b_K[n0 + n]
                bkvc_arr[64:128, n] = b_V[n0 + n]
            m["bqc"] = bqc_arr
            m["bkvc"] = bkvc_arr
        if use_g1:
            m["g1"] = ln1_g
        if use_b1:
            m["b1"] = ln1_b
        if use_g2:
            m["g2"] = ln2_g
        if use_b2:
            m["b2"] = ln2_b
        if n_mtiles:
            m["maskt"] = maskt_arr
        in_maps.append(m)

    res = bass_utils.run_bass_kernel_spmd(nc, in_maps,
                                          core_ids=list(range(NCORES)))

    out = np.zeros((B, SQ, D), dtype=np.float32)
    k_full = np.empty((B, SQ, N, H), dtype=np.float32)
    v_full = np.empty((B, SQ, N, H), dtype=np.float32)
    for core in range(NCORES):
        b = core // CPB
        n0 = (core % CPB) * HPC
        r = res.results[core]
        out[b] += r["outp"]
        k_full[b][:, n0:n0 + HPC, :] = r["ko"]
        v_full[b][:, n0:n0 + HPC, :] = r["vo"]
    out += b_O
    return out, k_full, v_full
